# revision 19
# baseline (speedup 1.0000x reference)
"""nn_AttentionC Trainium2 kernel (8 NeuronCores, SPMD).

Sharding: h-axis (64) split into 8 chunks of 8 rows, one per core; each core's
x slab is host-padded to [b2, t10, h10, w72] fp16 tokens (conv zero-padding
baked in). Only cross-core traffic: AllReduce of per-(b,head) [48,48] q/k
gram matrices (110 KB).

Per core (PSUM fp32):
  qkv 1x1 conv on PE -> padded slabs (q/k channels quantized to fp8e4 x16,
  v channels fp16); depthwise 3x3x3:
    q/k: fp8 DoubleRow diag matmuls, two taps per matmul (taps (dt,-1,dw) and
         (dt,+1,dw) differ by 144 B in the slab = 16-aligned pair stride),
         3.6x fewer PE cycles than fp16 diag taps; softmax+normalize washes
         out the fp8 error (measured 5e-4 overall).
    v:   fp16 diag taps (fp8 on the v path fails the 2e-2 gate).
  q~,k~ transposed on PE -> [q;k] grams on PE -> AllReduce -> batched
  norm/softmax on DVE/ACT -> block-diag attn @ v on PE -> proj 1x1 conv on
  PE -> fp32 out.
"""
import numpy as np

DIM = 192
HEADS = 8
HD = DIM // HEADS  # 24
B, T, H, W = 2, 8, 8, 64  # per-core owned h rows = 8
HP, TP = 10, 10
XW = 66  # x staging row width (wpad1 + 64 + wpad1)
WP = 72  # slab row pitch: 64->72 so dh +/-1 tap pairs are 16B apart (fp8)
SLAB = HP * WP  # 720
NTOK = B * T * H * W  # 8192 owned tokens per core
NCORES = 8
C3 = 3 * DIM
NPADTOK = B * TP * HP * XW  # 13200 (x staging tokens, 66-wide rows)
ASCALE = 16.0  # fp8 slab scale
WSCALE = 64.0  # fp8 diag scale
DW_DEQ = 1.0 / (ASCALE * WSCALE)

_CACHE = {}

MTILES = [(0, 128), (128, 128), (256, 128), (384, 128), (512, 64)]
KTILES = [(0, 128), (128, 64)]
TAPS = [(dt, dh, dw) for dt in (-1, 0, 1) for dh in (-1, 0, 1)
        for dw in (-1, 0, 1)]
# fp8 DoubleRow pair plan for q/k: per dt-plane, 6 matmuls: j in 0..2 pair
# taps (dt,-1,dw=j-1)+(dt,+1,dw=j-1); j in 3..5 are singles (dt,0,dw=j-4)
# with a zeroed second slot.
NQK_TILES = 6  # per (mtile, dt-plane)
# v taps computed off the PE: DVE does the per-channel multiply
# (tensor_scalar, 4x perf mode at fp16), Pool does the accumulate add
# (tensor_tensor).  First entry is the accumulator init and must be dt=0
# (never dropped at t boundaries).
OFF_ORDER = [(0, 0, 0), (0, -1, -1), (0, -1, 1), (0, 1, -1), (0, 1, 1),
             (0, -1, 0), (0, 1, 0), (0, 0, -1), (0, 0, 1),
             (-1, 0, 0), (1, 0, 0), (-1, -1, 0)]
N_OFF = 12
OFFTAPS = set(OFF_ORDER[:N_OFF])
QKCONV8 = True  # q/k half of the 1x1 conv in fp8 DoubleRow (single term)


def _build():
    import concourse.bacc as bacc
    import concourse.mybir as mybir
    import concourse.tile as tile
    from concourse import masks
    from concourse.ap import AP
    import bass_rust

    F32 = mybir.dt.float32
    F16 = mybir.dt.float16
    F8 = mybir.dt.float8e4
    AL = mybir.AluOpType
    AF = mybir.ActivationFunctionType
    AX = mybir.AxisListType
    DR = mybir.MatmulPerfMode.DoubleRow

    nc = bacc.Bacc("TRN2", target_bir_lowering=False, debug=False,
                   num_devices=NCORES)

    x16 = nc.dram_tensor("x16", [DIM, NPADTOK], F16, kind="ExternalInput").ap()
    # fp8 copy of x, 192 channels as 2 k-tiles of 96 in the same partitions
    # (DoubleRow contraction for the q/k half of the 1x1 conv)
    x8 = nc.dram_tensor("x8", [96, 2 * NPADTOK], F8, kind="ExternalInput").ap()
    # q/k 1x1 conv weights fp8 [96, 2, 128] per qk mtile
    wq8d = nc.dram_tensor("wq8d", [96, 3 * 256], F8, kind="ExternalInput").ap()
    qkvwT = nc.dram_tensor("qkvwT", [DIM, C3], F16, kind="ExternalInput").ap()
    qkvb = nc.dram_tensor("qkvb", [128, 5], F32, kind="ExternalInput").ap()
    # per-channel f32 v-diag values for the Pool-engine taps [128, 2*27]
    vdws = nc.dram_tensor("vdws", [128, 54], F32, kind="ExternalInput").ap()
    # fp8 DoubleRow diag-pair tiles for q/k: 3 mtiles x 3 planes x 6 tiles,
    # each [128, 2, 128] fp8 (values 64*d on the diagonal)
    qkdiag = nc.dram_tensor("qkdiag", [128, 3 * 3 * NQK_TILES * 256], F8,
                            kind="ExternalInput").ap()
    # fp16 exact diag tiles for v (mtile 3: 128ch, mtile 4: 64ch)
    vdiag3 = nc.dram_tensor("vdiag3", [128, 27 * 128], F16,
                            kind="ExternalInput").ap()
    vdiag4 = nc.dram_tensor("vdiag4", [64, 27 * 64], F16,
                            kind="ExternalInput").ap()
    dwb = nc.dram_tensor("dwb", [128, 5], F32, kind="ExternalInput").ap()
    # proj weight transposed, [192 k, 192 m] fp16
    projwT = nc.dram_tensor("projwT", [DIM, DIM], F16, kind="ExternalInput").ap()
    projb = nc.dram_tensor("projb", [128, 2], F32, kind="ExternalInput").ap()
    temp = nc.dram_tensor("temp", [16, 1], F32, kind="ExternalInput").ap()
    out = nc.dram_tensor("out", [DIM, NTOK], F32, kind="ExternalOutput").ap()

    gram_in = nc.dram_tensor("gram_in", [16, 48, 48], F32).ap()
    gram_out = nc.dram_tensor("gram_out", [16, 48, 48], F32,
                              addr_space="Shared").ap()
    attn_dram = nc.dram_tensor("attn_dram", [16, HD, HD], F16).ap()

    with tile.TileContext(nc) as tc:
        with (
            tc.tile_pool(name="wp", bufs=1) as wp,
            tc.tile_pool(name="xp", bufs=4) as xp,
            tc.tile_pool(name="qslab", bufs=5) as slp,
            tc.tile_pool(name="qk", bufs=1) as qkpool,
            tc.tile_pool(name="ev", bufs=4) as ev,
            tc.tile_pool(name="small", bufs=1) as sp,
            tc.tile_pool(name="ps", bufs=4, space="PSUM") as psp,
            tc.tile_pool(name="pst", bufs=2, space="PSUM") as pst,
            tc.tile_pool(name="psg", bufs=1, space="PSUM") as psg,
        ):
            # ---------------- weights ----------------
            wq = []
            for ko, kc in KTILES:
                t = wp.tile([kc, C3], F16, tag=f"wq{ko}")
                nc.sync.dma_start(out=t[:], in_=qkvwT[ko:ko + kc, :])
                wq.append(t)
            wq8 = wp.tile([96, 3 * 256], F8, tag="wq8")
            nc.sync.dma_start(out=wq8[:], in_=wq8d)
            vdws_s = wp.tile([128, 54], F32, tag="vdws")
            nc.sync.dma_start(out=vdws_s[:], in_=vdws)
            qkvb_s = wp.tile([128, 5], F32, tag="qkvb")
            nc.sync.dma_start(out=qkvb_s[:], in_=qkvb)
            qkdiag_s = wp.tile([128, 3 * 3 * NQK_TILES * 256], F8,
                               tag="qkdiag")
            nc.sync.dma_start(out=qkdiag_s[:], in_=qkdiag)
            vd3 = wp.tile([128, 27 * 128], F16, tag="vd3")
            nc.sync.dma_start(out=vd3[:], in_=vdiag3)
            vd4 = wp.tile([64, 27 * 64], F16, tag="vd4")
            nc.sync.dma_start(out=vd4[:], in_=vdiag4)
            dwb_s = wp.tile([128, 5], F32, tag="dwb")
            nc.sync.dma_start(out=dwb_s[:], in_=dwb)
            wproj = []
            for ki, (ko, kc) in enumerate(KTILES):
                t = wp.tile([kc, DIM], F16, tag=f"wproj{ki}")
                nc.sync.dma_start(out=t[:], in_=projwT[ko:ko + kc, :])
                wproj.append(t)
            projb_s = wp.tile([128, 2], F32, tag="projb")
            nc.sync.dma_start(out=projb_s[:], in_=projb)
            temp_s = wp.tile([16, 1], F32, tag="temp")
            nc.sync.dma_start(out=temp_s[:], in_=temp)

            ident16 = wp.tile([128, 128], F16, tag="ident16")
            masks.make_identity(nc, ident16[:])

            # dw outputs: q~,k~ (384 ch) in 3 tiles, v (192 ch) in 2 tiles
            qk_t = [qkpool.tile([128, NTOK], F16, tag=f"qk{i}", name=f"qk{i}")
                    for i in range(3)]
            v_t = [qkpool.tile([vc, NTOK], F16, tag=f"v{i}", name=f"v{i}")
                   for i, vc in enumerate([128, 64])]

            # ---------------- qkv conv + depthwise ----------------
            # x staging rows are 66 wide; slab rows are 72 wide (alignment
            # pad).  conv output written as [5 rows x 66] halves; slab cols
            # 66..71 of each row are memset once per slab.
            def qkv_slab(b, t_, slabs):
                    xoff = (b * TP + t_) * HP * XW
                    xt = []
                    for ko, kc in KTILES:
                        xx = xp.tile([kc, HP * XW], F16, tag=f"x{ko}")
                        nc.sync.dma_start(
                            out=xx[:],
                            in_=x16[ko:ko + kc, xoff:xoff + HP * XW])
                        xt.append(xx)
                    if QKCONV8:
                        # slot pitch 672 (16-aligned) holding 660 data cols
                        xx8 = xp.tile([96, 2, 672], F8, tag="x8")
                        nc.sync.dma_start(
                            out=xx8[:, :, 0:HP * XW],
                            in_=x8.rearrange("p (i n) -> p i n",
                                             i=2)[:, :, xoff:xoff + HP * XW])
                    mts = []
                    for mi, (mo, mc) in enumerate(MTILES):
                        sl = slp.tile([mc, SLAB], F8 if mi < 3 else F16,
                                      tag=f"sl{mi}")
                        slr = sl.rearrange("p (h w) -> p h w", h=HP)
                        nc.gpsimd.memset(slr[:, :, XW:WP], 0.0)
                        for half in range(2):
                            ps = psp.tile([128, 512], F32, tag="mm")
                            if mi < 3 and QKCONV8:
                                pd = xx8.ap[0]
                                rhs = AP(tensor=xx8.tensor,
                                         offset=xx8.offset + 330 * half,
                                         ap=bass_rust.VecI64Pair(
                                             [[pd[0], pd[1]],
                                              [672, 2], [1, 330]]))
                                nc.tensor.matmul(
                                    ps[:mc, :330],
                                    wq8[:, 256 * mi:256 * (mi + 1)]
                                    .rearrange("p (i m) -> p i m", i=2),
                                    rhs, start=True, stop=True, perf_mode=DR)
                            else:
                                for ki, (ko, kc) in enumerate(KTILES):
                                    nc.tensor.matmul(
                                        ps[:mc, :330],
                                        wq[ki][:, mo:mo + mc],
                                        xt[ki][:, 330 * half:330 * (half + 1)],
                                        start=(ki == 0), stop=(ki == 1))
                            nc.scalar.activation(
                                slr[:, 5 * half:5 * (half + 1), 0:XW],
                                ps[:mc, :330].rearrange(
                                    "p (h w) -> p h w", h=5),
                                AF.Identity, bias=qkvb_s[:mc, mi:mi + 1],
                                scale=ASCALE if mi < 3 else 1.0)
                        mts.append(sl)
                    slabs[t_] = mts

            def pair_rhs(src, offA, delta):
                """[128, 2, 4, 64] view of the fp8 slab: slot i at
                offA+i*delta, then 4 rows of 64 at pitch WP."""
                pd = src.ap[0]
                return AP(tensor=src.tensor, offset=src.offset + offA,
                          ap=bass_rust.VecI64Pair(
                              [[pd[0], pd[1]], [delta, 2], [WP, 4], [1, 64]]))

            def win(src, row, dwv, mc):
                """[mc, 4, 64] window of a slab at given start row/w shift."""
                return src[:mc].rearrange(
                    "p (h w) -> p h w", h=HP)[:, row:row + 4,
                                              1 + dwv:65 + dwv]

            def dw_chunk(b, t_o, slabs):
                  for half in range(2):
                    chunk = (b * T + t_o) * 2 + half
                    co = 256 * chunk
                    planes = [dt for dt in (-1, 0, 1)
                              if not ((t_o == 0 and dt == -1) or
                                      (t_o == T - 1 and dt == 1))]
                    # q/k mtiles: fp8 DoubleRow, 6 matmuls per plane
                    for mi in range(3):
                        ps = psp.tile([128, 512], F32, tag="mm")
                        nmm = len(planes) * NQK_TILES
                        j = 0
                        for dt in planes:
                            src = slabs[t_o + 1 + dt][mi]
                            for jj in range(NQK_TILES):
                                dwv = jj % 3 - 1
                                if jj < 3:  # pair (dh=-1)+(dh=+1)
                                    offA = (4 * half) * WP + 1 + dwv
                                    delta = 2 * WP
                                else:  # single (dh=0), slot B zero-weighted
                                    offA = (4 * half + 1) * WP + 1 + dwv
                                    delta = 2 * WP if half == 0 else -2 * WP
                                ti = (mi * 3 + (dt + 1)) * NQK_TILES + jj
                                nc.tensor.matmul(
                                    ps[:128, :256],
                                    qkdiag_s[:, 256 * ti:256 * (ti + 1)]
                                    .rearrange("p (i m) -> p i m", i=2),
                                    pair_rhs(src, offA, delta),
                                    start=(j == 0), stop=(j == nmm - 1),
                                    perf_mode=DR)
                                j += 1
                        nc.scalar.activation(
                            qk_t[mi][:, co:co + 256], ps[:128, :256],
                            AF.Identity, bias=dwb_s[:128, mi:mi + 1],
                            scale=DW_DEQ)
                    # v mtiles: exact fp16 diag taps on PE, OFFTAPS on Pool
                    keep = [(ti, tap) for ti, tap in enumerate(TAPS)
                            if tap not in OFFTAPS and
                            not ((t_o == 0 and tap[0] == -1) or
                                 (t_o == T - 1 and tap[0] == 1))]
                    offk = [tap for tap in OFF_ORDER[:N_OFF]
                            if not ((t_o == 0 and tap[0] == -1) or
                                    (t_o == T - 1 and tap[0] == 1))]
                    for vi, (mi, vd) in enumerate(((3, vd3), (4, vd4))):
                        mc = MTILES[mi][1]
                        # offloaded taps: DVE multiply (+bias on the first),
                        # Pool accumulate
                        acc = ev.tile([mc, 256], F16, tag=f"vacc{vi}",
                                      name=f"acc{vi}")
                        for oj, (dt, dh, dwv) in enumerate(offk):
                            ti = TAPS.index((dt, dh, dwv))
                            dcol = vdws_s[:mc, 27 * vi + ti:27 * vi + ti + 1]
                            w_in = win(slabs[t_o + 1 + dt][mi],
                                       4 * half + 1 + dh, dwv, mc)
                            if oj == 0:
                                nc.vector.tensor_scalar(
                                    acc[:].rearrange("p (h w) -> p h w", h=4),
                                    w_in, dcol, dwb_s[:mc, mi:mi + 1],
                                    AL.mult, AL.add)
                            else:
                                tmp = ev.tile([mc, 256], F16,
                                              tag=f"vtmp{vi}",
                                              name=f"tmp{vi}")
                                nc.vector.tensor_scalar(
                                    tmp[:].rearrange("p (h w) -> p h w", h=4),
                                    w_in, dcol, None, AL.mult)
                                nc.gpsimd.tensor_tensor(
                                    acc[:], acc[:], tmp[:], AL.add)
                        ps = psp.tile([128, 512], F32, tag="mm")
                        for j, (ti, (dt, dh, dwv)) in enumerate(keep):
                            src = slabs[t_o + 1 + dt][mi]
                            nc.tensor.matmul(
                                ps[:mc, :256], vd[:, mc * ti:mc * (ti + 1)],
                                win(src, 4 * half + 1 + dh, dwv, mc),
                                start=(j == 0), stop=(j == len(keep) - 1))
                        nc.vector.scalar_tensor_tensor(
                            v_t[vi][:, co:co + 256], acc[:], 1.0,
                            ps[:mc, :256], AL.mult, AL.add)

            gps = [psg.tile([48, 384], F32, tag=f"gram{i}", name=f"gram{i}") for i in range(2)]
            for b in range(B):
                slabs = {}
                for t_ in (1, 2, 3):
                    qkv_slab(b, t_, slabs)
                for t_o in range(T):
                    dw_chunk(b, t_o, slabs)
                    if t_o + 4 <= T:
                        qkv_slab(b, t_o + 4, slabs)
                # transposes + grams for this batch, then its AllReduce --
                # batch 0's collective overlaps batch 1's depthwise work
                for c64 in range(32 * b, 32 * (b + 1)):
                    qkT = ev.tile([128, 384], F16, tag="qkT")
                    for i in range(3):
                        tps = pst.tile([128, 128], F16, tag="trps")
                        nc.tensor.transpose(
                            tps[:], qk_t[i][:, 128 * c64:128 * (c64 + 1)],
                            ident16[:])
                        nc.vector.tensor_copy(qkT[:, 128 * i:128 * (i + 1)],
                                              tps[:])
                    for h in range(HEADS):
                        z = qkT[:, 48 * h:48 * (h + 1)]
                        nc.tensor.matmul(
                            gps[b][:, 48 * h:48 * (h + 1)], z, z,
                            start=(c64 % 32 == 0 and h == 0),
                            stop=(c64 % 32 == 31 and h == HEADS - 1))
                gs = ev.tile([48, 384], F32, tag="gs")
                nc.vector.tensor_copy(gs[:], gps[b][:])
                nc.sync.dma_start(
                    out=gram_in[8 * b:8 * (b + 1)].rearrange(
                        "g c d -> c g d"),
                    in_=gs[:].rearrange("c (g d) -> c g d", g=8))
                nc.gpsimd.collective_compute(
                    "AllReduce", AL.add,
                    replica_groups=[list(range(NCORES))],
                    ins=[gram_in[8 * b:8 * (b + 1)]],
                    outs=[gram_out[8 * b:8 * (b + 1)]])

            # ---------------- norms + softmax (batched [16, .]) -------------
            qq_f = sp.tile([16, 576], F32, tag="qqf")
            kk_f = sp.tile([16, 576], F32, tag="kkf")
            qk_f = sp.tile([16, 576], F32, tag="qkf")
            nc.sync.dma_start(
                out=qq_f[:].rearrange("p (c d) -> p c d", c=24),
                in_=gram_out[:, 0:24, 0:24])
            nc.sync.dma_start(
                out=kk_f[:].rearrange("p (c d) -> p c d", c=24),
                in_=gram_out[:, 24:48, 24:48])
            nc.sync.dma_start(
                out=qk_f[:].rearrange("p (c d) -> p c d", c=24),
                in_=gram_out[:, 0:24, 24:48])

            dm = sp.tile([16, 576], F32, tag="dm")
            nc.gpsimd.memset(dm[:], 0.0)
            nc.gpsimd.affine_select(
                out=dm[:], in_=dm[:], compare_op=AL.not_equal, fill=1.0,
                base=0, pattern=[[1, 24], [-1, 24]], channel_multiplier=0)

            def diag_rsqrt(src, tag):
                t1 = sp.tile([16, 576], F32, tag=tag + "a")
                nc.vector.tensor_mul(t1[:], src[:], dm[:])
                n2 = sp.tile([16, 24], F32, tag=tag + "b")
                nc.vector.tensor_reduce(
                    n2[:], t1[:].rearrange("p (c d) -> p c d", c=24),
                    axis=AX.X, op=AL.add)
                nrm = sp.tile([16, 24], F32, tag=tag + "c")
                nc.scalar.sqrt(nrm[:], n2[:])
                nc.vector.tensor_scalar_max(nrm[:], nrm[:], 1e-12)
                r = sp.tile([16, 24], F32, tag=tag + "d")
                nc.vector.reciprocal(r[:], nrm[:])
                return r

            rq = diag_rsqrt(qq_f, "rq")
            rk = diag_rsqrt(kk_f, "rk")

            a1 = sp.tile([16, 576], F32, tag="a1")
            nc.vector.tensor_mul(
                a1[:].rearrange("p (c d) -> p c d", c=24),
                qk_f[:].rearrange("p (c d) -> p c d", c=24),
                rq[:].rearrange("p (c one) -> p c one", one=1).broadcast_to(
                    (16, 24, 24)))
            nc.vector.tensor_mul(
                a1[:].rearrange("p (c d) -> p c d", c=24),
                a1[:].rearrange("p (c d) -> p c d", c=24),
                rk[:].rearrange("p (one d) -> p one d", one=1).broadcast_to(
                    (16, 24, 24)))
            nc.vector.tensor_scalar_mul(a1[:], a1[:], temp_s[:])

            mx = sp.tile([16, 24], F32, tag="mx")
            nc.vector.tensor_reduce(
                mx[:], a1[:].rearrange("p (c d) -> p c d", c=24),
                axis=AX.X, op=AL.max)
            nc.vector.tensor_sub(
                a1[:].rearrange("p (c d) -> p c d", c=24),
                a1[:].rearrange("p (c d) -> p c d", c=24),
                mx[:].rearrange("p (c one) -> p c one", one=1).broadcast_to(
                    (16, 24, 24)))
            ex = sp.tile([16, 576], F32, tag="ex")
            nc.scalar.activation(ex[:], a1[:], AF.Exp)
            sm = sp.tile([16, 24], F32, tag="sm")
            nc.vector.tensor_reduce(
                sm[:], ex[:].rearrange("p (c d) -> p c d", c=24),
                axis=AX.X, op=AL.add)
            rs = sp.tile([16, 24], F32, tag="rs")
            nc.vector.reciprocal(rs[:], sm[:])
            at16 = sp.tile([16, 576], F16, tag="at16")
            nc.vector.tensor_mul(
                at16[:].rearrange("p (c d) -> p c d", c=24),
                ex[:].rearrange("p (c d) -> p c d", c=24),
                rs[:].rearrange("p (c one) -> p c one", one=1).broadcast_to(
                    (16, 24, 24)))
            nc.sync.dma_start(
                out=attn_dram[:],
                in_=at16[:].rearrange("p (c d) -> p c d", c=24))

            # block-diag attn^T per batch, split into k-tiles 128+64
            bd = []
            for b in range(B):
                bts = []
                for ki, (ko, kc) in enumerate(KTILES):
                    bdt = sp.tile([kc, DIM], F16, tag=f"bd{b}_{ki}")
                    nc.vector.memset(bdt[:], 0.0)
                    bts.append(bdt)
                for h in range(HEADS):
                    src_a = attn_dram[8 * b + h].rearrange("c d -> d c")
                    r0, r1 = HD * h, HD * (h + 1)
                    if r1 <= 128:
                        nc.sync.dma_start(
                            out=bts[0][r0:r1, r0:r1], in_=src_a)
                    elif r0 >= 128:
                        nc.sync.dma_start(
                            out=bts[1][r0 - 128:r1 - 128, r0:r1], in_=src_a)
                    else:  # h == 5: rows 120..144 straddle the k-tile split
                        nc.sync.dma_start(
                            out=bts[0][r0:128, r0:r1], in_=src_a[0:128 - r0])
                        nc.sync.dma_start(
                            out=bts[1][0:r1 - 128, r0:r1],
                            in_=src_a[128 - r0:HD])
                bd.append(bts)

            # ---------------- attn@v + proj ----------------
            for chunk in range(B * T):
                b = chunk // T
                co = 512 * chunk
                aos = [ev.tile([kc, 512], F16, tag=f"ao{ki}", name=f"ao{ki}")
                       for ki, (ko, kc) in enumerate(KTILES)]
                for mi, (mo, mc) in enumerate(KTILES):
                    ps = psp.tile([128, 512], F32, tag="mm")
                    for ki in range(2):
                        nc.tensor.matmul(
                            ps[:mc, :], bd[b][ki][:, mo:mo + mc],
                            v_t[ki][:, co:co + 512],
                            start=(ki == 0), stop=(ki == 1))
                    nc.vector.tensor_copy(aos[mi][:, :], ps[:mc, :])
                for mi, (mo, mc) in enumerate(KTILES):
                    ps = psp.tile([128, 512], F32, tag="mm")
                    for ki in range(2):
                        nc.tensor.matmul(
                            ps[:mc, :], wproj[ki][:, mo:mo + mc],
                            aos[ki][:, :],
                            start=(ki == 0), stop=(ki == 1))
                    of = ev.tile([128, 512], F32, tag="of")
                    nc.vector.tensor_scalar(
                        of[:mc, :], ps[:mc, :], projb_s[:mc, mi:mi + 1],
                        None, AL.add)
                    nc.sync.dma_start(out=out[mo:mo + mc, co:co + 512],
                                      in_=of[:mc, :])
    nc.compile()
    return nc


def _prep_inputs(x, qkv_w, qkv_b, dw_w, dw_b, temperature, proj_w, proj_b):
    """Host-side prep: per-core padded fp16 slabs + shared weights."""
    x = np.asarray(x, np.float32)
    b_, c_, t_, h_, w_ = x.shape  # 2, 192, 8, 64, 64
    qkv_w2 = np.asarray(qkv_w, np.float32).reshape(C3, DIM)
    dw_w2 = np.asarray(dw_w, np.float32).reshape(C3, 27)
    proj_w2 = np.asarray(proj_w, np.float32).reshape(DIM, DIM)
    # permute qkv channels: [q_h0, k_h0, q_h1, k_h1, ..., v] so each head's
    # (q,k) columns are adjacent after transpose (contiguous gram operands)
    perm = []
    for h in range(HEADS):
        perm.extend(range(HD * h, HD * (h + 1)))          # q_h
        perm.extend(range(DIM + HD * h, DIM + HD * (h + 1)))  # k_h
    perm.extend(range(2 * DIM, 3 * DIM))                  # v unchanged
    perm = np.array(perm)
    qkv_w2 = qkv_w2[perm]
    dw_w2 = dw_w2[perm]
    qkv_b = np.asarray(qkv_b, np.float32)[perm]
    dw_b = np.asarray(dw_b, np.float32)[perm]

    import ml_dtypes
    FP8 = ml_dtypes.float8_e4m3

    wqT = np.ascontiguousarray(qkv_w2.T).astype(np.float16)  # [192, 576]
    qkvb_h = np.zeros((128, 5), np.float32)
    dwb_h = np.zeros((128, 5), np.float32)
    for mi, (mo, mc) in enumerate(MTILES):
        s = ASCALE if mi < 3 else 1.0  # qk slab evicted as fp8(ASCALE*psum)
        qkvb_h[:mc, mi] = np.asarray(qkv_b, np.float32)[mo:mo + mc] * s
        dwb_h[:mc, mi] = np.asarray(dw_b, np.float32)[mo:mo + mc]

    # fp8 DoubleRow diag-pair tiles for q/k (values WSCALE*d, fp8-rounded)
    tap_i = {tap: i for i, tap in enumerate(TAPS)}
    qkd = np.zeros((128, 3 * 3 * NQK_TILES * 256), FP8)
    d8 = (WSCALE * dw_w2).astype(FP8)  # [576, 27]
    rng = np.arange(128)
    for mi in range(3):
        mo = 128 * mi
        for dti, dt in enumerate((-1, 0, 1)):
            for jj in range(NQK_TILES):
                dwv = jj % 3 - 1
                ti = (mi * 3 + dti) * NQK_TILES + jj
                base = 256 * ti
                if jj < 3:
                    qkd[rng, base + rng] = d8[mo + rng, tap_i[(dt, -1, dwv)]]
                    qkd[rng, base + 128 + rng] = d8[mo + rng,
                                                    tap_i[(dt, 1, dwv)]]
                else:
                    qkd[rng, base + rng] = d8[mo + rng, tap_i[(dt, 0, dwv)]]

    # exact fp16 diag tiles for v
    vd3_h = np.zeros((128, 27 * 128), np.float16)
    vd4_h = np.zeros((64, 27 * 64), np.float16)
    for ti in range(27):
        vd3_h[rng, 128 * ti + rng] = dw_w2[384 + rng, ti].astype(np.float16)
        r64 = np.arange(64)
        vd4_h[r64, 64 * ti + r64] = dw_w2[512 + r64, ti].astype(np.float16)

    # per-channel f32 v-diag columns for the Pool-engine taps
    vdws_h = np.zeros((128, 54), np.float32)
    vdws_h[:, 0:27] = dw_w2[384:512]
    vdws_h[:64, 27:54] = dw_w2[512:576]

    # fp8 q/k 1x1-conv weights [96, 2, 128] per qk mtile (lhsT layout:
    # W[p, j, m] = qkv_w[out=mo+m, in=p+96j])
    wq8_h = np.zeros((96, 3 * 256), FP8)
    for mi in range(3):
        for j in range(2):
            blk = qkv_w2[128 * mi:128 * (mi + 1), 96 * j:96 * (j + 1)].T
            wq8_h[:, 256 * mi + 128 * j:256 * mi + 128 * (j + 1)] = \
                blk.astype(FP8)
    # proj lhsT with contraction padded 192->2x96 (no padding needed: 96*2)
    projwT_h = np.ascontiguousarray(proj_w2.T).astype(np.float16)  # [192,192]
    projb_h = np.zeros((128, 2), np.float32)
    projb_h[:128, 0] = np.asarray(proj_b, np.float32)[0:128]
    projb_h[:64, 1] = np.asarray(proj_b, np.float32)[128:192]
    temp_h = np.tile(np.asarray(temperature, np.float32).reshape(HEADS),
                     2).reshape(16, 1)  # g = b*8+h

    in_maps = []
    for i in range(NCORES):
        # padded slab [b, t10, h10, w66], h rows 8i-1 .. 8i+9 clamped->zero
        xs = np.zeros((b_, TP, HP, XW, c_), np.float32)
        hlo, hhi = 8 * i - 1, 8 * i + 9
        slo, shi = max(0, hlo), min(h_, hhi)
        # x [b,c,t,h,w] -> [b,t,h,w,c]
        xt = x[:, :, :, slo:shi, :].transpose(0, 2, 3, 4, 1)
        xs[:, 1:9, (slo - hlo):(slo - hlo) + (shi - slo), 1:65, :] = xt
        xflat = xs.reshape(b_ * TP * HP * XW, c_)
        x16 = np.ascontiguousarray(xflat.T).astype(np.float16)
        x8_h = np.ascontiguousarray(
            xflat.T.reshape(2, 96, NPADTOK).transpose(1, 0, 2)
            .reshape(96, 2 * NPADTOK)).astype(FP8)
        in_maps.append({
            "x16": x16, "x8": x8_h, "wq8d": wq8_h, "vdws": vdws_h,
            "qkvwT": wqT, "qkvb": qkvb_h, "qkdiag": qkd,
            "vdiag3": vd3_h, "vdiag4": vd4_h,
            "dwb": dwb_h, "projwT": projwT_h, "projb": projb_h,
            "temp": temp_h,
        })
    return in_maps


def _get_runner():
    """Build once; return a persistent sharded-jit callable (the per-call
    closure in bass2jax.run_bass_via_pjrt defeats jax's jit cache)."""
    if "runner" in _CACHE:
        return _CACHE["runner"]
    import jax
    for flag, val in [("jax_compilation_cache_dir", "/tmp/jax_kernel_cache"),
                      ("jax_persistent_cache_min_compile_time_secs", 1.0),
                      ("jax_persistent_cache_min_entry_size_bytes", 0)]:
        try:
            jax.config.update(flag, val)
        except Exception:
            pass
    import jax.numpy as jnp
    from jax.sharding import Mesh, PartitionSpec
    from jax.experimental.shard_map import shard_map
    import concourse.mybir as mybir
    from concourse import bass2jax

    nc = _build()
    bass2jax.install_neuronx_cc_hook()

    partition_name = (nc.partition_id_tensor.name
                      if nc.partition_id_tensor else None)
    in_names, out_names, out_avals, zero_shapes = [], [], [], []
    for alloc in nc.m.functions[0].allocations:
        if not isinstance(alloc, mybir.MemoryLocationSet):
            continue
        name = alloc.memorylocations[0].name
        if alloc.kind == "ExternalInput":
            if name != partition_name:
                in_names.append(name)
        elif alloc.kind == "ExternalOutput":
            shape = tuple(alloc.tensor_shape)
            dtype = mybir.dt.np(alloc.dtype)
            out_names.append(name)
            out_avals.append(jax.core.ShapedArray(shape, dtype))
            zero_shapes.append((shape, dtype))
    n_params = len(in_names)
    all_names = in_names + out_names
    if partition_name is not None:
        all_names.append(partition_name)

    def _body(*args):
        operands = list(args)
        if partition_name is not None:
            operands.append(bass2jax.partition_id_tensor())
        outs = bass2jax._bass_exec_p.bind(
            *operands, out_avals=tuple(out_avals), in_names=tuple(all_names),
            out_names=tuple(out_names), lowering_input_output_aliases=(),
            sim_require_finite=True, sim_require_nnan=True, nc=nc)
        return tuple(outs)

    devices = jax.devices()[:NCORES]
    mesh = Mesh(np.asarray(devices), ("core",))
    n_outs = len(out_names)
    sharded = jax.jit(
        shard_map(_body, mesh=mesh,
                  in_specs=(PartitionSpec("core"),) * (n_params + n_outs),
                  out_specs=(PartitionSpec("core"),) * n_outs,
                  check_rep=False),
        donate_argnums=tuple(range(n_params, n_params + n_outs)),
        keep_unused=True)

    def run(in_maps):
        concat_in = [np.concatenate([in_maps[c][nm] for c in range(NCORES)],
                                    axis=0) for nm in in_names]
        concat_zeros = [np.zeros((NCORES * s[0], *s[1:]), dt)
                        for s, dt in zero_shapes]
        out_arrs = sharded(*concat_in, *concat_zeros)
        return [
            {nm: np.asarray(out_arrs[i]).reshape(NCORES, *out_avals[i].shape)[c]
             for i, nm in enumerate(out_names)}
            for c in range(NCORES)]

    _CACHE["runner"] = run
    return run


def kernel(x, qkv_w, qkv_b, dw_w, dw_b, temperature, proj_w, proj_b):
    run = _get_runner()
    in_maps = _prep_inputs(x, qkv_w, qkv_b, dw_w, dw_b, temperature,
                           proj_w, proj_b)
    results = run(in_maps)
    b_, c_, t_, h_, w_ = np.asarray(x).shape
    outf = np.empty((b_, c_, t_, h_, w_), np.float32)
    for i in range(NCORES):
        o = results[i]["out"].reshape(c_, b_, t_, H, w_)
        outf[:, :, :, 8 * i:8 * i + 8, :] = o.transpose(1, 0, 2, 3, 4)
    return outf



# revision 30
# speedup vs baseline: 1.6491x; 1.6491x over previous
"""nn_AttentionC Trainium2 kernel (8 NeuronCores, SPMD).

Sharding: h-axis (64) split into 8 chunks of 8 rows, one per core; each core's
x slab is host-padded to [b2, t10, h10, w72] fp16 tokens (conv zero-padding
baked in). Only cross-core traffic: AllReduce of per-(b,head) [48,48] q/k
gram matrices (110 KB).

Per core (PSUM fp32):
  qkv 1x1 conv on PE -> padded slabs (q/k channels quantized to fp8e4 x16,
  v channels fp16); depthwise 3x3x3:
    q/k: fp8 DoubleRow diag matmuls, two taps per matmul (taps (dt,-1,dw) and
         (dt,+1,dw) differ by 144 B in the slab = 16-aligned pair stride),
         3.6x fewer PE cycles than fp16 diag taps; softmax+normalize washes
         out the fp8 error (measured 5e-4 overall).
    v:   fp16 diag taps (fp8 on the v path fails the 2e-2 gate).
  q~,k~ transposed on PE -> [q;k] grams on PE -> AllReduce -> batched
  norm/softmax on DVE/ACT -> block-diag attn @ v on PE -> proj 1x1 conv on
  PE -> fp32 out.
"""
import numpy as np

DIM = 192
HEADS = 8
HD = DIM // HEADS  # 24
B, T, H, W = 2, 8, 8, 64  # per-core owned h rows = 8
HP, TP = 10, 10
XW = 66  # x staging row width (wpad1 + 64 + wpad1)
WP = 72  # slab row pitch: 64->72 so dh +/-1 tap pairs are 16B apart (fp8)
SLAB = HP * WP  # 720
NTOK = B * T * H * W  # 8192 owned tokens per core
NCORES = 8
C3 = 3 * DIM
NPADTOK = B * TP * HP * XW  # 13200 (x staging tokens, 66-wide rows)
ASCALE = 16.0  # fp8 slab scale
WSCALE = 64.0  # fp8 diag scale
DW_DEQ = 1.0 / (ASCALE * WSCALE)

_CACHE = {}

MTILES = [(0, 128), (128, 128), (256, 128), (384, 128), (512, 64)]
KTILES = [(0, 128), (128, 64)]
TAPS = [(dt, dh, dw) for dt in (-1, 0, 1) for dh in (-1, 0, 1)
        for dw in (-1, 0, 1)]
# fp8 DoubleRow pair plan for q/k: per dt-plane, 6 matmuls: j in 0..2 pair
# taps (dt,-1,dw=j-1)+(dt,+1,dw=j-1); j in 3..5 are singles (dt,0,dw=j-4)
# with a zeroed second slot.
NQK_TILES = 6  # per (mtile, dt-plane)
# v taps computed off the PE: DVE does the per-channel multiply
# (tensor_scalar, 4x perf mode at fp16), Pool does the accumulate add
# (tensor_tensor).  First entry is the accumulator init and must be dt=0
# (never dropped at t boundaries).
OFF_ORDER = [(0, 0, 0), (0, -1, -1), (0, -1, 1), (0, 1, -1), (0, 1, 1),
             (0, -1, 0), (0, 1, 0), (0, 0, -1), (0, 0, 1),
             (-1, 0, 0), (1, 0, 0), (-1, -1, 0)]
N_OFF = 10
OFFTAPS = set(OFF_ORDER[:N_OFF])
QKCONV8 = True  # q/k half of the 1x1 conv in fp8 DoubleRow (single term)


def _build():
    import concourse.bacc as bacc
    import concourse.mybir as mybir
    import concourse.tile as tile
    from concourse import masks
    from concourse.ap import AP
    import bass_rust

    F32 = mybir.dt.float32
    F16 = mybir.dt.float16
    F8 = mybir.dt.float8e4
    AL = mybir.AluOpType
    AF = mybir.ActivationFunctionType
    AX = mybir.AxisListType
    DR = mybir.MatmulPerfMode.DoubleRow

    nc = bacc.Bacc("TRN2", target_bir_lowering=False, debug=False,
                   num_devices=NCORES)

    x16 = nc.dram_tensor("x16", [DIM, NPADTOK], F16, kind="ExternalInput").ap()
    # fp8 copy of x, 192 channels as 2 k-tiles of 96 in the same partitions
    # (DoubleRow contraction for the q/k half of the 1x1 conv)
    x8 = nc.dram_tensor("x8", [96, 2 * NPADTOK], F8, kind="ExternalInput").ap()
    # q/k 1x1 conv weights fp8 [96, 2, 128] per qk mtile
    wq8d = nc.dram_tensor("wq8d", [96, 3 * 256], F8, kind="ExternalInput").ap()
    qkvwT = nc.dram_tensor("qkvwT", [DIM, C3], F16, kind="ExternalInput").ap()
    qkvb = nc.dram_tensor("qkvb", [128, 5], F32, kind="ExternalInput").ap()
    # per-channel f32 v-diag values for the Pool-engine taps [128, 2*27]
    vdws = nc.dram_tensor("vdws", [128, 54], F32, kind="ExternalInput").ap()
    # fp8 DoubleRow diag-pair tiles for q/k: 3 mtiles x 3 planes x 6 tiles,
    # each [128, 2, 128] fp8 (values 64*d on the diagonal)
    qkdiag = nc.dram_tensor("qkdiag", [128, 3 * 3 * NQK_TILES * 256], F8,
                            kind="ExternalInput").ap()
    # fp16 exact diag tiles for v (mtile 3: 128ch, mtile 4: 64ch)
    vdiag3 = nc.dram_tensor("vdiag3", [128, 27 * 128], F16,
                            kind="ExternalInput").ap()
    vdiag4 = nc.dram_tensor("vdiag4", [64, 27 * 64], F16,
                            kind="ExternalInput").ap()
    dwb = nc.dram_tensor("dwb", [128, 5], F32, kind="ExternalInput").ap()
    # proj weight transposed, [192 k, 192 m] fp16
    projwT = nc.dram_tensor("projwT", [DIM, DIM], F16, kind="ExternalInput").ap()
    projb = nc.dram_tensor("projb", [128, 2], F32, kind="ExternalInput").ap()
    temp = nc.dram_tensor("temp", [16, 1], F32, kind="ExternalInput").ap()
    out = nc.dram_tensor("out", [DIM, NTOK], F32, kind="ExternalOutput").ap()

    gram_in = nc.dram_tensor("gram_in", [16, 48, 48], F32).ap()
    gram_out = nc.dram_tensor("gram_out", [16, 48, 48], F32,
                              addr_space="Shared").ap()
    attn_dram = nc.dram_tensor("attn_dram", [16, HD, HD], F16).ap()

    with tile.TileContext(nc) as tc:
        with (
            tc.tile_pool(name="wp", bufs=1) as wp,
            tc.tile_pool(name="xp", bufs=3) as xp,
            tc.tile_pool(name="qslab", bufs=5) as slp,
            tc.tile_pool(name="qk", bufs=1) as qkpool,
            tc.tile_pool(name="ev", bufs=3) as ev,
            tc.tile_pool(name="small", bufs=1) as sp,
            tc.tile_pool(name="ps", bufs=4, space="PSUM") as psp,
            tc.tile_pool(name="psg", bufs=1, space="PSUM") as psg,
        ):
            # ---------------- weights ----------------
            wq = []
            for ko, kc in KTILES:
                t = wp.tile([kc, C3], F16, tag=f"wq{ko}")
                nc.sync.dma_start(out=t[:], in_=qkvwT[ko:ko + kc, :])
                wq.append(t)
            wq8 = wp.tile([96, 3 * 256], F8, tag="wq8")
            nc.sync.dma_start(out=wq8[:], in_=wq8d)
            vdws_s = wp.tile([128, 54], F32, tag="vdws")
            nc.sync.dma_start(out=vdws_s[:], in_=vdws)
            qkvb_s = wp.tile([128, 5], F32, tag="qkvb")
            nc.sync.dma_start(out=qkvb_s[:], in_=qkvb)
            qkdiag_s = wp.tile([128, 3 * 3 * NQK_TILES * 256], F8,
                               tag="qkdiag")
            nc.sync.dma_start(out=qkdiag_s[:], in_=qkdiag)
            vd3 = wp.tile([128, 27 * 128], F16, tag="vd3")
            nc.sync.dma_start(out=vd3[:], in_=vdiag3)
            vd4 = wp.tile([64, 27 * 64], F16, tag="vd4")
            nc.sync.dma_start(out=vd4[:], in_=vdiag4)
            dwb_s = wp.tile([128, 5], F32, tag="dwb")
            nc.sync.dma_start(out=dwb_s[:], in_=dwb)
            wproj = []
            for ki, (ko, kc) in enumerate(KTILES):
                t = wp.tile([kc, DIM], F16, tag=f"wproj{ki}")
                nc.sync.dma_start(out=t[:], in_=projwT[ko:ko + kc, :])
                wproj.append(t)
            projb_s = wp.tile([128, 2], F32, tag="projb")
            nc.sync.dma_start(out=projb_s[:], in_=projb)
            temp_s = wp.tile([16, 1], F32, tag="temp")
            nc.sync.dma_start(out=temp_s[:], in_=temp)

            ident16 = wp.tile([128, 128], F16, tag="ident16")
            masks.make_identity(nc, ident16[:])

            # dw outputs: v (192 ch) in 2 materialized tiles; q~/k~ go
            # through per-chunk ring tiles + XBAR DMA transpose into
            # qkT_all [tok128, chunk64, ch384]
            v_t = [qkpool.tile([vc, NTOK], F16, tag=f"v{i}", name=f"v{i}")
                   for i, vc in enumerate([128, 64])]
            qkT_all = qkpool.tile([128, 64, 384], F16, tag="qkT_all",
                                  name="qkT_all")

            # ---------------- qkv conv + depthwise ----------------
            # x staging rows are 66 wide; slab rows are 72 wide (alignment
            # pad).  conv output written as [5 rows x 66] halves; slab cols
            # 66..71 of each row are memset once per slab.
            def qkv_slab(b, t_, slabs):
                    xoff = (b * TP + t_) * HP * XW
                    xt = []
                    for ko, kc in KTILES:
                        xx = xp.tile([kc, HP * XW], F16, tag=f"x{ko}")
                        nc.sync.dma_start(
                            out=xx[:],
                            in_=x16[ko:ko + kc, xoff:xoff + HP * XW])
                        xt.append(xx)
                    if QKCONV8:
                        # slot pitch 672 (16-aligned) holding 660 data cols
                        xx8 = xp.tile([96, 2, 672], F8, tag="x8")
                        nc.sync.dma_start(
                            out=xx8[:, :, 0:HP * XW],
                            in_=x8.rearrange("p (i n) -> p i n",
                                             i=2)[:, :, xoff:xoff + HP * XW])
                    mts = []
                    for mi, (mo, mc) in enumerate(MTILES):
                        sl = slp.tile([mc, SLAB], F8 if mi < 3 else F16,
                                      tag=f"sl{mi}")
                        slr = sl.rearrange("p (h w) -> p h w", h=HP)
                        nc.gpsimd.memset(slr[:, :, XW:WP], 0.0)
                        for half in range(2):
                            ps = psp.tile([128, 512], F32, tag="mm")
                            if mi < 3 and QKCONV8:
                                pd = xx8.ap[0]
                                rhs = AP(tensor=xx8.tensor,
                                         offset=xx8.offset + 330 * half,
                                         ap=bass_rust.VecI64Pair(
                                             [[pd[0], pd[1]],
                                              [672, 2], [1, 330]]))
                                nc.tensor.matmul(
                                    ps[:mc, :330],
                                    wq8[:, 256 * mi:256 * (mi + 1)]
                                    .rearrange("p (i m) -> p i m", i=2),
                                    rhs, start=True, stop=True, perf_mode=DR)
                            else:
                                for ki, (ko, kc) in enumerate(KTILES):
                                    nc.tensor.matmul(
                                        ps[:mc, :330],
                                        wq[ki][:, mo:mo + mc],
                                        xt[ki][:, 330 * half:330 * (half + 1)],
                                        start=(ki == 0), stop=(ki == 1))
                            nc.scalar.activation(
                                slr[:, 5 * half:5 * (half + 1), 0:XW],
                                ps[:mc, :330].rearrange(
                                    "p (h w) -> p h w", h=5),
                                AF.Identity, bias=qkvb_s[:mc, mi:mi + 1],
                                scale=ASCALE if mi < 3 else 1.0)
                        mts.append(sl)
                    slabs[t_] = mts

            def pair_rhs(src, offA, delta):
                """[128, 2, 8, 64] view of the fp8 slab: slot i at
                offA+i*delta, then 8 rows of 64 at pitch WP."""
                pd = src.ap[0]
                return AP(tensor=src.tensor, offset=src.offset + offA,
                          ap=bass_rust.VecI64Pair(
                              [[pd[0], pd[1]], [delta, 2], [WP, 8], [1, 64]]))

            def win(src, row, dwv, mc):
                """[mc, 8, 64] window of a slab at given start row/w shift."""
                return src[:mc].rearrange(
                    "p (h w) -> p h w", h=HP)[:, row:row + 8,
                                              1 + dwv:65 + dwv]

            def dw_chunk(b, t_o, slabs):
                    chunk = b * T + t_o
                    co = 512 * chunk
                    planes = [dt for dt in (-1, 0, 1)
                              if not ((t_o == 0 and dt == -1) or
                                      (t_o == T - 1 and dt == 1))]
                    # q/k mtiles: fp8 DoubleRow, 6 matmuls per plane,
                    # full 512-token chunk per matmul
                    for mi in range(3):
                        ps = psp.tile([128, 512], F32, tag="mm")
                        nmm = len(planes) * NQK_TILES
                        j = 0
                        for dt in planes:
                            src = slabs[t_o + 1 + dt][mi]
                            for jj in range(NQK_TILES):
                                dwv = jj % 3 - 1
                                if jj < 3:  # pair (dh=-1)+(dh=+1)
                                    offA = 1 + dwv
                                    delta = 2 * WP
                                else:  # single (dh=0): slot B zero-weighted,
                                    # reads 16 cols over (finite, in-bounds)
                                    offA = WP + 1 + dwv
                                    delta = 16
                                ti = (mi * 3 + (dt + 1)) * NQK_TILES + jj
                                nc.tensor.matmul(
                                    ps[:128, :512],
                                    qkdiag_s[:, 256 * ti:256 * (ti + 1)]
                                    .rearrange("p (i m) -> p i m", i=2),
                                    pair_rhs(src, offA, delta),
                                    start=(j == 0), stop=(j == nmm - 1),
                                    perf_mode=DR)
                                j += 1
                        qkc = ev.tile([128, 512], F16, tag=f"qkc{mi}",
                                      name=f"qkc{mi}")
                        nc.scalar.activation(
                            qkc[:], ps[:128, :512],
                            AF.Identity, bias=dwb_s[:128, mi:mi + 1],
                            scale=DW_DEQ)
                        qdma = (nc.sync, nc.scalar, nc.sync)[mi]
                        qdma.dma_start_transpose(
                            out=qkT_all[:, 4 * chunk:4 * (chunk + 1),
                                        128 * mi:128 * (mi + 1)],
                            in_=qkc[:])
                    # v mtiles: exact fp16 diag taps on PE, OFF_ORDER taps
                    # as fused MACs on DVE
                    keep = [(ti, tap) for ti, tap in enumerate(TAPS)
                            if tap not in OFFTAPS and
                            not ((t_o == 0 and tap[0] == -1) or
                                 (t_o == T - 1 and tap[0] == 1))]
                    offk = [tap for tap in OFF_ORDER[:N_OFF]
                            if not ((t_o == 0 and tap[0] == -1) or
                                    (t_o == T - 1 and tap[0] == 1))]
                    for vi, (mi, vd) in enumerate(((3, vd3), (4, vd4))):
                        mc = MTILES[mi][1]
                        acc = ev.tile([mc, 512], F16, tag=f"vacc{vi}",
                                      name=f"acc{vi}")
                        accv = acc[:].rearrange("p (h w) -> p h w", h=8)
                        for oj, (dt, dh, dwv) in enumerate(offk):
                            ti = TAPS.index((dt, dh, dwv))
                            dcol = vdws_s[:mc, 27 * vi + ti:27 * vi + ti + 1]
                            w_in = win(slabs[t_o + 1 + dt][mi],
                                       1 + dh, dwv, mc)
                            if oj == 0:
                                nc.vector.tensor_scalar(
                                    accv, w_in, dcol, dwb_s[:mc, mi:mi + 1],
                                    AL.mult, AL.add)
                            else:
                                nc.vector.scalar_tensor_tensor(
                                    accv, w_in, dcol, accv, AL.mult, AL.add)
                        ps = psp.tile([128, 512], F32, tag="mm")
                        for j, (ti, (dt, dh, dwv)) in enumerate(keep):
                            src = slabs[t_o + 1 + dt][mi]
                            nc.tensor.matmul(
                                ps[:mc, :512], vd[:, mc * ti:mc * (ti + 1)],
                                win(src, 1 + dh, dwv, mc),
                                start=(j == 0), stop=(j == len(keep) - 1))
                        nc.vector.scalar_tensor_tensor(
                            v_t[vi][:, co:co + 512], acc[:], 1.0,
                            ps[:mc, :512], AL.mult, AL.add)

            gps = [psg.tile([48, 384], F32, tag=f"gram{i}", name=f"gram{i}") for i in range(2)]
            for b in range(B):
                slabs = {}
                for t_ in (1, 2, 3):
                    qkv_slab(b, t_, slabs)
                for t_o in range(T):
                    dw_chunk(b, t_o, slabs)
                    if t_o + 4 <= T:
                        qkv_slab(b, t_o + 4, slabs)
                # grams for this batch (qkT_all filled by the per-chunk
                # transpose DMAs); batch 0's AllReduce overlaps batch 1's
                # depthwise work
                for c64 in range(32 * b, 32 * (b + 1)):
                    for h in range(HEADS):
                        z = qkT_all[:, c64, 48 * h:48 * (h + 1)]
                        nc.tensor.matmul(
                            gps[b][:, 48 * h:48 * (h + 1)], z, z,
                            start=(c64 % 32 == 0 and h == 0),
                            stop=(c64 % 32 == 31 and h == HEADS - 1))
                gs = ev.tile([48, 384], F32, tag="gs")
                nc.vector.tensor_copy(gs[:], gps[b][:])
                nc.sync.dma_start(
                    out=gram_in[8 * b:8 * (b + 1)].rearrange(
                        "g c d -> c g d"),
                    in_=gs[:].rearrange("c (g d) -> c g d", g=8))
                nc.gpsimd.collective_compute(
                    "AllReduce", AL.add,
                    replica_groups=[list(range(NCORES))],
                    ins=[gram_in[8 * b:8 * (b + 1)]],
                    outs=[gram_out[8 * b:8 * (b + 1)]])

            # ---------------- norms + softmax (batched [16, .]) -------------
            qq_f = sp.tile([16, 576], F32, tag="qqf")
            kk_f = sp.tile([16, 576], F32, tag="kkf")
            qk_f = sp.tile([16, 576], F32, tag="qkf")
            nc.sync.dma_start(
                out=qq_f[:].rearrange("p (c d) -> p c d", c=24),
                in_=gram_out[:, 0:24, 0:24])
            nc.sync.dma_start(
                out=kk_f[:].rearrange("p (c d) -> p c d", c=24),
                in_=gram_out[:, 24:48, 24:48])
            nc.sync.dma_start(
                out=qk_f[:].rearrange("p (c d) -> p c d", c=24),
                in_=gram_out[:, 0:24, 24:48])

            dm = sp.tile([16, 576], F32, tag="dm")
            nc.gpsimd.memset(dm[:], 0.0)
            nc.gpsimd.affine_select(
                out=dm[:], in_=dm[:], compare_op=AL.not_equal, fill=1.0,
                base=0, pattern=[[1, 24], [-1, 24]], channel_multiplier=0)

            def diag_rsqrt(src, tag):
                t1 = sp.tile([16, 576], F32, tag=tag + "a")
                nc.vector.tensor_mul(t1[:], src[:], dm[:])
                n2 = sp.tile([16, 24], F32, tag=tag + "b")
                nc.vector.tensor_reduce(
                    n2[:], t1[:].rearrange("p (c d) -> p c d", c=24),
                    axis=AX.X, op=AL.add)
                nrm = sp.tile([16, 24], F32, tag=tag + "c")
                nc.scalar.sqrt(nrm[:], n2[:])
                nc.vector.tensor_scalar_max(nrm[:], nrm[:], 1e-12)
                r = sp.tile([16, 24], F32, tag=tag + "d")
                nc.vector.reciprocal(r[:], nrm[:])
                return r

            rq = diag_rsqrt(qq_f, "rq")
            rk = diag_rsqrt(kk_f, "rk")

            a1 = sp.tile([16, 576], F32, tag="a1")
            nc.vector.tensor_mul(
                a1[:].rearrange("p (c d) -> p c d", c=24),
                qk_f[:].rearrange("p (c d) -> p c d", c=24),
                rq[:].rearrange("p (c one) -> p c one", one=1).broadcast_to(
                    (16, 24, 24)))
            nc.vector.tensor_mul(
                a1[:].rearrange("p (c d) -> p c d", c=24),
                a1[:].rearrange("p (c d) -> p c d", c=24),
                rk[:].rearrange("p (one d) -> p one d", one=1).broadcast_to(
                    (16, 24, 24)))
            nc.vector.tensor_scalar_mul(a1[:], a1[:], temp_s[:])

            mx = sp.tile([16, 24], F32, tag="mx")
            nc.vector.tensor_reduce(
                mx[:], a1[:].rearrange("p (c d) -> p c d", c=24),
                axis=AX.X, op=AL.max)
            nc.vector.tensor_sub(
                a1[:].rearrange("p (c d) -> p c d", c=24),
                a1[:].rearrange("p (c d) -> p c d", c=24),
                mx[:].rearrange("p (c one) -> p c one", one=1).broadcast_to(
                    (16, 24, 24)))
            ex = sp.tile([16, 576], F32, tag="ex")
            nc.scalar.activation(ex[:], a1[:], AF.Exp)
            sm = sp.tile([16, 24], F32, tag="sm")
            nc.vector.tensor_reduce(
                sm[:], ex[:].rearrange("p (c d) -> p c d", c=24),
                axis=AX.X, op=AL.add)
            rs = sp.tile([16, 24], F32, tag="rs")
            nc.vector.reciprocal(rs[:], sm[:])
            at16 = sp.tile([16, 576], F16, tag="at16")
            nc.vector.tensor_mul(
                at16[:].rearrange("p (c d) -> p c d", c=24),
                ex[:].rearrange("p (c d) -> p c d", c=24),
                rs[:].rearrange("p (c one) -> p c one", one=1).broadcast_to(
                    (16, 24, 24)))
            nc.sync.dma_start(
                out=attn_dram[:],
                in_=at16[:].rearrange("p (c d) -> p c d", c=24))

            # block-diag attn^T per batch, split into k-tiles 128+64
            bd = []
            for b in range(B):
                bts = []
                for ki, (ko, kc) in enumerate(KTILES):
                    bdt = sp.tile([kc, DIM], F16, tag=f"bd{b}_{ki}")
                    nc.vector.memset(bdt[:], 0.0)
                    bts.append(bdt)
                for h in range(HEADS):
                    src_a = attn_dram[8 * b + h].rearrange("c d -> d c")
                    r0, r1 = HD * h, HD * (h + 1)
                    if r1 <= 128:
                        nc.sync.dma_start(
                            out=bts[0][r0:r1, r0:r1], in_=src_a)
                    elif r0 >= 128:
                        nc.sync.dma_start(
                            out=bts[1][r0 - 128:r1 - 128, r0:r1], in_=src_a)
                    else:  # h == 5: rows 120..144 straddle the k-tile split
                        nc.sync.dma_start(
                            out=bts[0][r0:128, r0:r1], in_=src_a[0:128 - r0])
                        nc.sync.dma_start(
                            out=bts[1][0:r1 - 128, r0:r1],
                            in_=src_a[128 - r0:HD])
                bd.append(bts)

            # ---------------- attn@v + proj ----------------
            for chunk in range(B * T):
                b = chunk // T
                co = 512 * chunk
                aos = [ev.tile([kc, 512], F16, tag=f"ao{ki}", name=f"ao{ki}")
                       for ki, (ko, kc) in enumerate(KTILES)]
                for mi, (mo, mc) in enumerate(KTILES):
                    ps = psp.tile([128, 512], F32, tag="mm")
                    for ki in range(2):
                        nc.tensor.matmul(
                            ps[:mc, :], bd[b][ki][:, mo:mo + mc],
                            v_t[ki][:, co:co + 512],
                            start=(ki == 0), stop=(ki == 1))
                    nc.vector.tensor_copy(aos[mi][:, :], ps[:mc, :])
                for mi, (mo, mc) in enumerate(KTILES):
                    ps = psp.tile([128, 512], F32, tag="mm")
                    for ki in range(2):
                        nc.tensor.matmul(
                            ps[:mc, :], wproj[ki][:, mo:mo + mc],
                            aos[ki][:, :],
                            start=(ki == 0), stop=(ki == 1))
                    of = ev.tile([128, 512], F32, tag="of")
                    nc.vector.tensor_scalar(
                        of[:mc, :], ps[:mc, :], projb_s[:mc, mi:mi + 1],
                        None, AL.add)
                    nc.sync.dma_start(out=out[mo:mo + mc, co:co + 512],
                                      in_=of[:mc, :])
    nc.compile()
    return nc


def _prep_inputs(x, qkv_w, qkv_b, dw_w, dw_b, temperature, proj_w, proj_b):
    """Host-side prep: per-core padded fp16 slabs + shared weights."""
    x = np.asarray(x, np.float32)
    b_, c_, t_, h_, w_ = x.shape  # 2, 192, 8, 64, 64
    qkv_w2 = np.asarray(qkv_w, np.float32).reshape(C3, DIM)
    dw_w2 = np.asarray(dw_w, np.float32).reshape(C3, 27)
    proj_w2 = np.asarray(proj_w, np.float32).reshape(DIM, DIM)
    # permute qkv channels: [q_h0, k_h0, q_h1, k_h1, ..., v] so each head's
    # (q,k) columns are adjacent after transpose (contiguous gram operands)
    perm = []
    for h in range(HEADS):
        perm.extend(range(HD * h, HD * (h + 1)))          # q_h
        perm.extend(range(DIM + HD * h, DIM + HD * (h + 1)))  # k_h
    perm.extend(range(2 * DIM, 3 * DIM))                  # v unchanged
    perm = np.array(perm)
    qkv_w2 = qkv_w2[perm]
    dw_w2 = dw_w2[perm]
    qkv_b = np.asarray(qkv_b, np.float32)[perm]
    dw_b = np.asarray(dw_b, np.float32)[perm]

    import ml_dtypes
    FP8 = ml_dtypes.float8_e4m3

    wqT = np.ascontiguousarray(qkv_w2.T).astype(np.float16)  # [192, 576]
    qkvb_h = np.zeros((128, 5), np.float32)
    dwb_h = np.zeros((128, 5), np.float32)
    for mi, (mo, mc) in enumerate(MTILES):
        s = ASCALE if mi < 3 else 1.0  # qk slab evicted as fp8(ASCALE*psum)
        qkvb_h[:mc, mi] = np.asarray(qkv_b, np.float32)[mo:mo + mc] * s
        dwb_h[:mc, mi] = np.asarray(dw_b, np.float32)[mo:mo + mc]

    # fp8 DoubleRow diag-pair tiles for q/k (values WSCALE*d, fp8-rounded)
    tap_i = {tap: i for i, tap in enumerate(TAPS)}
    qkd = np.zeros((128, 3 * 3 * NQK_TILES * 256), FP8)
    d8 = (WSCALE * dw_w2).astype(FP8)  # [576, 27]
    rng = np.arange(128)
    for mi in range(3):
        mo = 128 * mi
        for dti, dt in enumerate((-1, 0, 1)):
            for jj in range(NQK_TILES):
                dwv = jj % 3 - 1
                ti = (mi * 3 + dti) * NQK_TILES + jj
                base = 256 * ti
                if jj < 3:
                    qkd[rng, base + rng] = d8[mo + rng, tap_i[(dt, -1, dwv)]]
                    qkd[rng, base + 128 + rng] = d8[mo + rng,
                                                    tap_i[(dt, 1, dwv)]]
                else:
                    qkd[rng, base + rng] = d8[mo + rng, tap_i[(dt, 0, dwv)]]

    # exact fp16 diag tiles for v
    vd3_h = np.zeros((128, 27 * 128), np.float16)
    vd4_h = np.zeros((64, 27 * 64), np.float16)
    for ti in range(27):
        vd3_h[rng, 128 * ti + rng] = dw_w2[384 + rng, ti].astype(np.float16)
        r64 = np.arange(64)
        vd4_h[r64, 64 * ti + r64] = dw_w2[512 + r64, ti].astype(np.float16)

    # per-channel f32 v-diag columns for the Pool-engine taps
    vdws_h = np.zeros((128, 54), np.float32)
    vdws_h[:, 0:27] = dw_w2[384:512]
    vdws_h[:64, 27:54] = dw_w2[512:576]

    # fp8 q/k 1x1-conv weights [96, 2, 128] per qk mtile (lhsT layout:
    # W[p, j, m] = qkv_w[out=mo+m, in=p+96j])
    wq8_h = np.zeros((96, 3 * 256), FP8)
    for mi in range(3):
        for j in range(2):
            blk = qkv_w2[128 * mi:128 * (mi + 1), 96 * j:96 * (j + 1)].T
            wq8_h[:, 256 * mi + 128 * j:256 * mi + 128 * (j + 1)] = \
                blk.astype(FP8)
    # proj lhsT with contraction padded 192->2x96 (no padding needed: 96*2)
    projwT_h = np.ascontiguousarray(proj_w2.T).astype(np.float16)  # [192,192]
    projb_h = np.zeros((128, 2), np.float32)
    projb_h[:128, 0] = np.asarray(proj_b, np.float32)[0:128]
    projb_h[:64, 1] = np.asarray(proj_b, np.float32)[128:192]
    temp_h = np.tile(np.asarray(temperature, np.float32).reshape(HEADS),
                     2).reshape(16, 1)  # g = b*8+h

    in_maps = []
    for i in range(NCORES):
        # padded slab [b, t10, h10, w66], h rows 8i-1 .. 8i+9 clamped->zero
        xs = np.zeros((b_, TP, HP, XW, c_), np.float32)
        hlo, hhi = 8 * i - 1, 8 * i + 9
        slo, shi = max(0, hlo), min(h_, hhi)
        # x [b,c,t,h,w] -> [b,t,h,w,c]
        xt = x[:, :, :, slo:shi, :].transpose(0, 2, 3, 4, 1)
        xs[:, 1:9, (slo - hlo):(slo - hlo) + (shi - slo), 1:65, :] = xt
        xflat = xs.reshape(b_ * TP * HP * XW, c_)
        x16 = np.ascontiguousarray(xflat.T).astype(np.float16)
        x8_h = np.ascontiguousarray(
            xflat.T.reshape(2, 96, NPADTOK).transpose(1, 0, 2)
            .reshape(96, 2 * NPADTOK)).astype(FP8)
        in_maps.append({
            "x16": x16, "x8": x8_h, "wq8d": wq8_h, "vdws": vdws_h,
            "qkvwT": wqT, "qkvb": qkvb_h, "qkdiag": qkd,
            "vdiag3": vd3_h, "vdiag4": vd4_h,
            "dwb": dwb_h, "projwT": projwT_h, "projb": projb_h,
            "temp": temp_h,
        })
    return in_maps


def _get_runner():
    """Build once; return a persistent sharded-jit callable (the per-call
    closure in bass2jax.run_bass_via_pjrt defeats jax's jit cache)."""
    if "runner" in _CACHE:
        return _CACHE["runner"]
    import jax
    for flag, val in [("jax_compilation_cache_dir", "/tmp/jax_kernel_cache"),
                      ("jax_persistent_cache_min_compile_time_secs", 1.0),
                      ("jax_persistent_cache_min_entry_size_bytes", 0)]:
        try:
            jax.config.update(flag, val)
        except Exception:
            pass
    import jax.numpy as jnp
    from jax.sharding import Mesh, PartitionSpec
    from jax.experimental.shard_map import shard_map
    import concourse.mybir as mybir
    from concourse import bass2jax

    nc = _build()
    bass2jax.install_neuronx_cc_hook()

    partition_name = (nc.partition_id_tensor.name
                      if nc.partition_id_tensor else None)
    in_names, out_names, out_avals, zero_shapes = [], [], [], []
    for alloc in nc.m.functions[0].allocations:
        if not isinstance(alloc, mybir.MemoryLocationSet):
            continue
        name = alloc.memorylocations[0].name
        if alloc.kind == "ExternalInput":
            if name != partition_name:
                in_names.append(name)
        elif alloc.kind == "ExternalOutput":
            shape = tuple(alloc.tensor_shape)
            dtype = mybir.dt.np(alloc.dtype)
            out_names.append(name)
            out_avals.append(jax.core.ShapedArray(shape, dtype))
            zero_shapes.append((shape, dtype))
    n_params = len(in_names)
    all_names = in_names + out_names
    if partition_name is not None:
        all_names.append(partition_name)

    def _body(*args):
        operands = list(args)
        if partition_name is not None:
            operands.append(bass2jax.partition_id_tensor())
        outs = bass2jax._bass_exec_p.bind(
            *operands, out_avals=tuple(out_avals), in_names=tuple(all_names),
            out_names=tuple(out_names), lowering_input_output_aliases=(),
            sim_require_finite=True, sim_require_nnan=True, nc=nc)
        return tuple(outs)

    devices = jax.devices()[:NCORES]
    mesh = Mesh(np.asarray(devices), ("core",))
    n_outs = len(out_names)
    sharded = jax.jit(
        shard_map(_body, mesh=mesh,
                  in_specs=(PartitionSpec("core"),) * (n_params + n_outs),
                  out_specs=(PartitionSpec("core"),) * n_outs,
                  check_rep=False),
        donate_argnums=tuple(range(n_params, n_params + n_outs)),
        keep_unused=True)

    def run(in_maps):
        concat_in = [np.concatenate([in_maps[c][nm] for c in range(NCORES)],
                                    axis=0) for nm in in_names]
        concat_zeros = [np.zeros((NCORES * s[0], *s[1:]), dt)
                        for s, dt in zero_shapes]
        out_arrs = sharded(*concat_in, *concat_zeros)
        return [
            {nm: np.asarray(out_arrs[i]).reshape(NCORES, *out_avals[i].shape)[c]
             for i, nm in enumerate(out_names)}
            for c in range(NCORES)]

    _CACHE["runner"] = run
    return run


def kernel(x, qkv_w, qkv_b, dw_w, dw_b, temperature, proj_w, proj_b):
    run = _get_runner()
    in_maps = _prep_inputs(x, qkv_w, qkv_b, dw_w, dw_b, temperature,
                           proj_w, proj_b)
    results = run(in_maps)
    b_, c_, t_, h_, w_ = np.asarray(x).shape
    outf = np.empty((b_, c_, t_, h_, w_), np.float32)
    for i in range(NCORES):
        o = results[i]["out"].reshape(c_, b_, t_, H, w_)
        outf[:, :, :, 8 * i:8 * i + 8, :] = o.transpose(1, 0, 2, 3, 4)
    return outf



# revision 42
# speedup vs baseline: 1.7977x; 1.0901x over previous
"""nn_AttentionC Trainium2 kernel (8 NeuronCores, SPMD).

Sharding: h-axis (64) split into 8 chunks of 8 rows, one per core; each core's
x slab is host-padded to [b2, t10, h10, w72] fp16 tokens (conv zero-padding
baked in). Only cross-core traffic: AllReduce of per-(b,head) [48,48] q/k
gram matrices (110 KB).

Per core (PSUM fp32):
  qkv 1x1 conv on PE -> padded slabs (q/k channels quantized to fp8e4 x16,
  v channels fp16); depthwise 3x3x3:
    q/k: fp8 DoubleRow diag matmuls, two taps per matmul (taps (dt,-1,dw) and
         (dt,+1,dw) differ by 144 B in the slab = 16-aligned pair stride),
         3.6x fewer PE cycles than fp16 diag taps; softmax+normalize washes
         out the fp8 error (measured 5e-4 overall).
    v:   fp16 diag taps (fp8 on the v path fails the 2e-2 gate).
  q~,k~ transposed on PE -> [q;k] grams on PE -> AllReduce -> batched
  norm/softmax on DVE/ACT -> block-diag attn @ v on PE -> proj 1x1 conv on
  PE -> fp32 out.
"""
import numpy as np

DIM = 192
HEADS = 8
HD = DIM // HEADS  # 24
B, T, H, W = 2, 8, 8, 64  # per-core owned h rows = 8
HP, TP = 10, 10
XW = 66  # x staging row width (wpad1 + 64 + wpad1)
WP = 72  # slab row pitch: 64->72 so dh +/-1 tap pairs are 16B apart (fp8)
SLAB = HP * WP  # 720
NTOK = B * T * H * W  # 8192 owned tokens per core
NCORES = 8
C3 = 3 * DIM
NPADTOK = B * TP * HP * XW  # 13200 (x staging tokens, 66-wide rows)
ASCALE = 16.0  # fp8 slab scale
WSCALE = 64.0  # fp8 diag scale
DW_DEQ = 1.0 / (ASCALE * WSCALE)

_CACHE = {}

MTILES = [(0, 128), (128, 128), (256, 128), (384, 128), (512, 64)]
KTILES = [(0, 128), (128, 64)]
TAPS = [(dt, dh, dw) for dt in (-1, 0, 1) for dh in (-1, 0, 1)
        for dw in (-1, 0, 1)]
# fp8 DoubleRow pair plan for q/k: per dt-plane, 6 matmuls: j in 0..2 pair
# taps (dt,-1,dw=j-1)+(dt,+1,dw=j-1); j in 3..5 are singles (dt,0,dw=j-4)
# with a zeroed second slot.
NQK_TILES = 6  # per (mtile, dt-plane)
# v taps computed off the PE: DVE does the per-channel multiply
# (tensor_scalar, 4x perf mode at fp16), Pool does the accumulate add
# (tensor_tensor).  First entry is the accumulator init and must be dt=0
# (never dropped at t boundaries).
OFF_ORDER = [(0, 0, 0), (0, -1, -1), (0, -1, 1), (0, 1, -1), (0, 1, 1),
             (0, -1, 0), (0, 1, 0), (0, 0, -1), (0, 0, 1),
             (-1, 0, 0), (1, 0, 0), (-1, -1, 0)]
N_OFF = 10
OFFTAPS = set(OFF_ORDER[:N_OFF])
QKCONV8 = True  # q/k half of the 1x1 conv in fp8 DoubleRow (single term)


def _build():
    import concourse.bacc as bacc
    import concourse.mybir as mybir
    import concourse.tile as tile
    from concourse import masks
    from concourse.ap import AP
    import bass_rust

    F32 = mybir.dt.float32
    F16 = mybir.dt.float16
    F8 = mybir.dt.float8e4
    AL = mybir.AluOpType
    AF = mybir.ActivationFunctionType
    AX = mybir.AxisListType
    DR = mybir.MatmulPerfMode.DoubleRow

    nc = bacc.Bacc("TRN2", target_bir_lowering=False, debug=False,
                   num_devices=NCORES)

    x16 = nc.dram_tensor("x16", [DIM, NPADTOK], F16, kind="ExternalInput").ap()
    # fp8 copy of x, 192 channels as 2 k-tiles of 96 in the same partitions
    # (DoubleRow contraction for the q/k half of the 1x1 conv)
    x8 = nc.dram_tensor("x8", [96, 2 * NPADTOK], F8, kind="ExternalInput").ap()
    # q/k 1x1 conv weights fp8 [96, 2, 128] per qk mtile
    wq8d = nc.dram_tensor("wq8d", [96, 3 * 256], F8, kind="ExternalInput").ap()
    qkvwT = nc.dram_tensor("qkvwT", [DIM, C3], F16, kind="ExternalInput").ap()
    qkvb = nc.dram_tensor("qkvb", [128, 5], F32, kind="ExternalInput").ap()
    # per-channel f32 v-diag values for the Pool-engine taps [128, 2*27]
    vdws = nc.dram_tensor("vdws", [128, 54], F32, kind="ExternalInput").ap()
    # fp8 DoubleRow diag-pair tiles for q/k: 3 mtiles x 3 planes x 6 tiles,
    # each [128, 2, 128] fp8 (values 64*d on the diagonal)
    qkdiag = nc.dram_tensor("qkdiag", [128, 3 * 3 * NQK_TILES * 256], F8,
                            kind="ExternalInput").ap()
    # fp16 exact diag tiles for v (mtile 3: 128ch, mtile 4: 64ch)
    vdiag3 = nc.dram_tensor("vdiag3", [128, 27 * 128], F16,
                            kind="ExternalInput").ap()
    vdiag4 = nc.dram_tensor("vdiag4", [64, 27 * 64], F16,
                            kind="ExternalInput").ap()
    dwb = nc.dram_tensor("dwb", [128, 5], F32, kind="ExternalInput").ap()
    # proj weight transposed, [192 k, 192 m] fp16
    projwT = nc.dram_tensor("projwT", [DIM, DIM], F16, kind="ExternalInput").ap()
    projb = nc.dram_tensor("projb", [128, 2], F32, kind="ExternalInput").ap()
    temp = nc.dram_tensor("temp", [8, 2], F32, kind="ExternalInput").ap()
    out = nc.dram_tensor("out", [DIM, NTOK], F32, kind="ExternalOutput").ap()

    gram_in = nc.dram_tensor("gram_in", [16, 48, 48], F32).ap()
    gram_out = nc.dram_tensor("gram_out", [16, 48, 48], F32,
                              addr_space="Shared").ap()
    attn_dram = nc.dram_tensor("attn_dram", [16, HD, HD], F16).ap()

    with tile.TileContext(nc) as tc:
        with (
            tc.tile_pool(name="wp", bufs=1) as wp,
            tc.tile_pool(name="xp", bufs=3) as xp,
            tc.tile_pool(name="qslab", bufs=5) as slp,
            tc.tile_pool(name="qk", bufs=1) as qkpool,
            tc.tile_pool(name="ev", bufs=3) as ev,
            tc.tile_pool(name="small", bufs=1) as sp,
            tc.tile_pool(name="ps", bufs=4, space="PSUM") as psp,
            tc.tile_pool(name="psav", bufs=2, space="PSUM") as psav,
            tc.tile_pool(name="psg", bufs=1, space="PSUM") as psg,
        ):
            # ---------------- weights ----------------
            wq = []
            for ko, kc in KTILES:
                t = wp.tile([kc, C3], F16, tag=f"wq{ko}")
                nc.sync.dma_start(out=t[:], in_=qkvwT[ko:ko + kc, :])
                wq.append(t)
            wq8 = wp.tile([96, 3 * 256], F8, tag="wq8")
            nc.sync.dma_start(out=wq8[:], in_=wq8d)
            vdws_s = wp.tile([128, 54], F32, tag="vdws")
            nc.sync.dma_start(out=vdws_s[:], in_=vdws)
            qkvb_s = wp.tile([128, 5], F32, tag="qkvb")
            nc.sync.dma_start(out=qkvb_s[:], in_=qkvb)
            qkdiag_s = wp.tile([128, 3 * 3 * NQK_TILES * 256], F8,
                               tag="qkdiag")
            nc.scalar.dma_start(out=qkdiag_s[:], in_=qkdiag)
            vd3 = wp.tile([128, 27 * 128], F16, tag="vd3")
            nc.scalar.dma_start(out=vd3[:], in_=vdiag3)
            vd4 = wp.tile([64, 27 * 64], F16, tag="vd4")
            nc.scalar.dma_start(out=vd4[:], in_=vdiag4)
            dwb_s = wp.tile([128, 5], F32, tag="dwb")
            nc.sync.dma_start(out=dwb_s[:], in_=dwb)
            wproj = []
            for ki, (ko, kc) in enumerate(KTILES):
                t = wp.tile([kc, DIM], F16, tag=f"wproj{ki}")
                nc.sync.dma_start(out=t[:], in_=projwT[ko:ko + kc, :])
                wproj.append(t)
            projb_s = wp.tile([128, 2], F32, tag="projb")
            nc.sync.dma_start(out=projb_s[:], in_=projb)
            temp_s = wp.tile([8, 2], F32, tag="temp")
            nc.sync.dma_start(out=temp_s[:], in_=temp)

            ident16 = wp.tile([128, 128], F16, tag="ident16")
            masks.make_identity(nc, ident16[:])

            # dw outputs: v (192 ch) in 2 materialized tiles; q~/k~ go
            # through per-chunk ring tiles + XBAR DMA transpose into
            # qkT_all [tok128, chunk64, ch384]
            v_t = [qkpool.tile([vc, NTOK], F16, tag=f"v{i}", name=f"v{i}")
                   for i, vc in enumerate([128, 64])]
            qkT_all = qkpool.tile([128, 64, 384], F16, tag="qkT_all",
                                  name="qkT_all")

            # ---------------- qkv conv + depthwise ----------------
            # x staging rows are 66 wide; slab rows are 72 wide (alignment
            # pad).  conv output written as [5 rows x 66] halves; slab cols
            # 66..71 of each row are memset once per slab.
            def qkv_slab(b, t_, slabs):
                    xoff = (b * TP + t_) * HP * XW
                    xt = []
                    for ko, kc in KTILES:
                        xx = xp.tile([kc, HP * XW], F16, tag=f"x{ko}")
                        nc.sync.dma_start(
                            out=xx[:],
                            in_=x16[ko:ko + kc, xoff:xoff + HP * XW])
                        xt.append(xx)
                    if QKCONV8:
                        # slot pitch 672 (16-aligned) holding 660 data cols
                        xx8 = xp.tile([96, 2, 672], F8, tag="x8")
                        nc.sync.dma_start(
                            out=xx8[:, :, 0:HP * XW],
                            in_=x8.rearrange("p (i n) -> p i n",
                                             i=2)[:, :, xoff:xoff + HP * XW])
                    mts = []
                    for mi, (mo, mc) in enumerate(MTILES):
                        sl = slp.tile([mc, SLAB], F8 if mi < 3 else F16,
                                      tag=f"sl{mi}")
                        slr = sl.rearrange("p (h w) -> p h w", h=HP)
                        nc.gpsimd.memset(slr[:, :, XW:WP], 0.0)
                        for half in range(2):
                            ps = psp.tile([128, 512], F32, tag="mm")
                            if mi < 3 and QKCONV8:
                                pd = xx8.ap[0]
                                rhs = AP(tensor=xx8.tensor,
                                         offset=xx8.offset + 330 * half,
                                         ap=bass_rust.VecI64Pair(
                                             [[pd[0], pd[1]],
                                              [672, 2], [1, 330]]))
                                nc.tensor.matmul(
                                    ps[:mc, :330],
                                    wq8[:, 256 * mi:256 * (mi + 1)]
                                    .rearrange("p (i m) -> p i m", i=2),
                                    rhs, start=True, stop=True, perf_mode=DR)
                            else:
                                for ki, (ko, kc) in enumerate(KTILES):
                                    nc.tensor.matmul(
                                        ps[:mc, :330],
                                        wq[ki][:, mo:mo + mc],
                                        xt[ki][:, 330 * half:330 * (half + 1)],
                                        start=(ki == 0), stop=(ki == 1))
                            nc.scalar.activation(
                                slr[:, 5 * half:5 * (half + 1), 0:XW],
                                ps[:mc, :330].rearrange(
                                    "p (h w) -> p h w", h=5),
                                AF.Identity, bias=qkvb_s[:mc, mi:mi + 1],
                                scale=ASCALE if mi < 3 else 1.0)
                        mts.append(sl)
                    slabs[t_] = mts

            def pair_rhs(src, offA, delta):
                """[128, 2, 8, 64] view of the fp8 slab: slot i at
                offA+i*delta, then 8 rows of 64 at pitch WP."""
                pd = src.ap[0]
                return AP(tensor=src.tensor, offset=src.offset + offA,
                          ap=bass_rust.VecI64Pair(
                              [[pd[0], pd[1]], [delta, 2], [WP, 8], [1, 64]]))

            def win(src, row, dwv, mc):
                """[mc, 8, 64] window of a slab at given start row/w shift."""
                return src[:mc].rearrange(
                    "p (h w) -> p h w", h=HP)[:, row:row + 8,
                                              1 + dwv:65 + dwv]

            def gram_chunk(b, chunk):
                for c64 in range(4 * chunk, 4 * (chunk + 1)):
                    for h in range(HEADS):
                        z = qkT_all[:, c64, 48 * h:48 * (h + 1)]
                        nc.tensor.matmul(
                            gps[b][:, 48 * h:48 * (h + 1)], z, z,
                            start=(c64 == 32 * b and h == 0),
                            stop=(c64 == 32 * b + 31 and h == HEADS - 1))

            def dw_chunk(b, t_o, slabs):
                    chunk = b * T + t_o
                    co = 512 * chunk
                    planes = [dt for dt in (-1, 0, 1)
                              if not ((t_o == 0 and dt == -1) or
                                      (t_o == T - 1 and dt == 1))]
                    # q/k mtiles: fp8 DoubleRow, 6 matmuls per plane,
                    # full 512-token chunk per matmul
                    for mi in range(3):
                        ps = psp.tile([128, 512], F32, tag="mm")
                        nmm = len(planes) * NQK_TILES
                        j = 0
                        for dt in planes:
                            src = slabs[t_o + 1 + dt][mi]
                            for jj in range(NQK_TILES):
                                dwv = jj % 3 - 1
                                if jj < 3:  # pair (dh=-1)+(dh=+1)
                                    offA = 1 + dwv
                                    delta = 2 * WP
                                else:  # single (dh=0): slot B zero-weighted,
                                    # reads 16 cols over (finite, in-bounds)
                                    offA = WP + 1 + dwv
                                    delta = 16
                                ti = (mi * 3 + (dt + 1)) * NQK_TILES + jj
                                nc.tensor.matmul(
                                    ps[:128, :512],
                                    qkdiag_s[:, 256 * ti:256 * (ti + 1)]
                                    .rearrange("p (i m) -> p i m", i=2),
                                    pair_rhs(src, offA, delta),
                                    start=(j == 0), stop=(j == nmm - 1),
                                    perf_mode=DR)
                                j += 1
                        qkc = ev.tile([128, 512], F16, tag=f"qkc{mi}",
                                      name=f"qkc{mi}")
                        nc.scalar.activation(
                            qkc[:], ps[:128, :512],
                            AF.Identity, bias=dwb_s[:128, mi:mi + 1],
                            scale=DW_DEQ)
                        qdma = (nc.sync, nc.scalar, nc.sync)[mi]
                        qdma.dma_start_transpose(
                            out=qkT_all[:, 4 * chunk:4 * (chunk + 1),
                                        128 * mi:128 * (mi + 1)],
                            in_=qkc[:])
                    # grams for the PREVIOUS chunk (its transpose DMAs have
                    # had a full chunk of time to land; PE is in-order so a
                    # not-yet-ready gram matmul would stall the dw stream)
                    if t_o > 0:
                        gram_chunk(b, chunk - 1)
                    # v mtiles: exact fp16 diag taps on PE, OFF_ORDER taps
                    # as fused MACs on DVE
                    keep = [(ti, tap) for ti, tap in enumerate(TAPS)
                            if tap not in OFFTAPS and
                            not ((t_o == 0 and tap[0] == -1) or
                                 (t_o == T - 1 and tap[0] == 1))]
                    offk = [tap for tap in OFF_ORDER[:N_OFF]
                            if not ((t_o == 0 and tap[0] == -1) or
                                    (t_o == T - 1 and tap[0] == 1))]
                    for vi, (mi, vd) in enumerate(((3, vd3), (4, vd4))):
                        mc = MTILES[mi][1]
                        acc = ev.tile([mc, 512], F16, tag=f"vacc{vi}",
                                      name=f"acc{vi}")
                        accv = acc[:].rearrange("p (h w) -> p h w", h=8)
                        for oj, (dt, dh, dwv) in enumerate(offk):
                            ti = TAPS.index((dt, dh, dwv))
                            dcol = vdws_s[:mc, 27 * vi + ti:27 * vi + ti + 1]
                            w_in = win(slabs[t_o + 1 + dt][mi],
                                       1 + dh, dwv, mc)
                            if oj == 0:
                                nc.vector.tensor_scalar(
                                    accv, w_in, dcol, dwb_s[:mc, mi:mi + 1],
                                    AL.mult, AL.add)
                            else:
                                nc.vector.scalar_tensor_tensor(
                                    accv, w_in, dcol, accv, AL.mult, AL.add)
                        ps = psp.tile([128, 512], F32, tag="mm")
                        for j, (ti, (dt, dh, dwv)) in enumerate(keep):
                            src = slabs[t_o + 1 + dt][mi]
                            nc.tensor.matmul(
                                ps[:mc, :512], vd[:, mc * ti:mc * (ti + 1)],
                                win(src, 1 + dh, dwv, mc),
                                start=(j == 0), stop=(j == len(keep) - 1))
                        nc.vector.scalar_tensor_tensor(
                            v_t[vi][:, co:co + 512], acc[:], 1.0,
                            ps[:mc, :512], AL.mult, AL.add)

            # per-batch norms + softmax + attn@v + proj (emitted after each
            # batch's AllReduce so batch 0's tail overlaps batch 1's dw)
            def attn_batch(b):
                qq_f = sp.tile([8, 576], F32, tag="qqf", name="qq_f")
                kk_f = sp.tile([8, 576], F32, tag="kkf", name="kk_f")
                qk_f = sp.tile([8, 576], F32, tag="qkf", name="qk_f")
                g8 = gram_out[8 * b:8 * (b + 1)]
                nc.sync.dma_start(
                    out=qq_f[:].rearrange("p (c d) -> p c d", c=24),
                    in_=g8[:, 0:24, 0:24])
                nc.sync.dma_start(
                    out=kk_f[:].rearrange("p (c d) -> p c d", c=24),
                    in_=g8[:, 24:48, 24:48])
                nc.sync.dma_start(
                    out=qk_f[:].rearrange("p (c d) -> p c d", c=24),
                    in_=g8[:, 24:48, 0:24])

                def diag_rsqrt(src, tag):
                    t1 = sp.tile([8, 576], F32, tag=tag + "a", name="t1")
                    nc.vector.tensor_mul(t1[:], src[:], dm[:])
                    n2 = sp.tile([8, 24], F32, tag=tag + "b", name="n2")
                    nc.vector.tensor_reduce(
                        n2[:], t1[:].rearrange("p (c d) -> p c d", c=24),
                        axis=AX.X, op=AL.add)
                    nrm = sp.tile([8, 24], F32, tag=tag + "c", name="nrm")
                    nc.scalar.sqrt(nrm[:], n2[:])
                    nc.vector.tensor_scalar_max(nrm[:], nrm[:], 1e-12)
                    r = sp.tile([8, 24], F32, tag=tag + "d", name="r")
                    nc.vector.reciprocal(r[:], nrm[:])
                    return r

                rq = diag_rsqrt(qq_f, "rq")
                rk = diag_rsqrt(kk_f, "rk")

                a1 = sp.tile([8, 576], F32, tag="a1", name="a1")
                nc.vector.tensor_mul(
                    a1[:].rearrange("p (d c) -> p d c", d=24),
                    qk_f[:].rearrange("p (d c) -> p d c", d=24),
                    rk[:].rearrange("p (d one) -> p d one",
                                    one=1).broadcast_to((8, 24, 24)))
                nc.vector.tensor_mul(
                    a1[:].rearrange("p (d c) -> p d c", d=24),
                    a1[:].rearrange("p (d c) -> p d c", d=24),
                    rq[:].rearrange("p (one c) -> p one c",
                                    one=1).broadcast_to((8, 24, 24)))
                nc.vector.tensor_scalar_mul(a1[:], a1[:],
                                            temp_s[:, b:b + 1])

                mx = sp.tile([8, 24], F32, tag="mx", name="mx")
                a1_cd = a1[:].rearrange("p (d c) -> p c d", d=24)
                nc.vector.tensor_reduce(mx[:], a1_cd, axis=AX.X, op=AL.max)
                nc.vector.tensor_sub(
                    a1_cd, a1_cd,
                    mx[:].rearrange("p (c one) -> p c one",
                                    one=1).broadcast_to((8, 24, 24)))
                ex = sp.tile([8, 576], F32, tag="ex", name="ex")
                nc.scalar.activation(ex[:], a1[:], AF.Exp)
                sm = sp.tile([8, 24], F32, tag="sm", name="sm")
                nc.vector.tensor_reduce(
                    sm[:], ex[:].rearrange("p (d c) -> p c d", d=24),
                    axis=AX.X, op=AL.add)
                rs = sp.tile([8, 24], F32, tag="rs", name="rs")
                nc.vector.reciprocal(rs[:], sm[:])
                at16 = sp.tile([8, 576], F16, tag="at16", name="at16")
                nc.vector.tensor_mul(
                    at16[:].rearrange("p (d c) -> p d c", d=24),
                    ex[:].rearrange("p (d c) -> p d c", d=24),
                    rs[:].rearrange("p (one c) -> p one c",
                                    one=1).broadcast_to((8, 24, 24)))
                nc.sync.dma_start(
                    out=attn_dram[8 * b:8 * (b + 1)],
                    in_=at16[:].rearrange("p (c d) -> p c d", c=24))

                # block-diag attn^T, split into k-tiles 128+64
                bts = []
                for ki, (ko, kc) in enumerate(KTILES):
                    bdt = sp.tile([kc, DIM], F16, tag=f"bd{b}_{ki}",
                                  name="bdt")
                    nc.gpsimd.memset(bdt[:], 0.0)
                    bts.append(bdt)
                for h in range(HEADS):
                    src_a = attn_dram[8 * b + h]
                    q = nc.sync if h % 2 == 0 else nc.scalar
                    r0, r1 = HD * h, HD * (h + 1)
                    if r1 <= 128:
                        q.dma_start(out=bts[0][r0:r1, r0:r1], in_=src_a)
                    elif r0 >= 128:
                        q.dma_start(
                            out=bts[1][r0 - 128:r1 - 128, r0:r1], in_=src_a)
                    else:  # h == 5: rows 120..144 straddle the k-tile split
                        q.dma_start(
                            out=bts[0][r0:128, r0:r1], in_=src_a[0:128 - r0])
                        q.dma_start(
                            out=bts[1][0:r1 - 128, r0:r1],
                            in_=src_a[128 - r0:HD])

                return bts

            def av_proj(bts, chunks):
                for chunk in chunks:
                    co = 512 * chunk
                    aos = [ev.tile([kc, 512], F16, tag=f"ao{ki}",
                                   name=f"ao{ki}")
                           for ki, (ko, kc) in enumerate(KTILES)]
                    for mi, (mo, mc) in enumerate(KTILES):
                        ps = psav.tile([128, 512], F32, tag="av")
                        for ki in range(2):
                            nc.tensor.matmul(
                                ps[:mc, :], bts[ki][:, mo:mo + mc],
                                v_t[ki][:, co:co + 512],
                                start=(ki == 0), stop=(ki == 1))
                        nc.scalar.activation(aos[mi][:, :], ps[:mc, :],
                                             AF.Identity)
                    for mi, (mo, mc) in enumerate(KTILES):
                        ps = psav.tile([128, 512], F32, tag="av")
                        for ki in range(2):
                            nc.tensor.matmul(
                                ps[:mc, :], wproj[ki][:, mo:mo + mc],
                                aos[ki][:, :],
                                start=(ki == 0), stop=(ki == 1))
                        of = ev.tile([128, 512], F32, tag="of")
                        nc.vector.tensor_scalar(
                            of[:mc, :], ps[:mc, :], projb_s[:mc, mi:mi + 1],
                            None, AL.add)
                        nc.sync.dma_start(out=out[mo:mo + mc, co:co + 512],
                                          in_=of[:mc, :])

            gps = [psg.tile([48, 384], F32, tag=f"gram{i}", name=f"gram{i}") for i in range(2)]
            dm = sp.tile([8, 576], F32, tag="dm")
            nc.gpsimd.memset(dm[:], 0.0)
            nc.gpsimd.affine_select(
                out=dm[:], in_=dm[:], compare_op=AL.not_equal, fill=1.0,
                base=0, pattern=[[1, 24], [-1, 24]], channel_multiplier=0)
            batch_work = []
            for b in range(B):
                slabs = {}
                for t_ in (1, 2, 3):
                    qkv_slab(b, t_, slabs)
                for t_o in range(T):
                    dw_chunk(b, t_o, slabs)
                    if t_o + 4 <= T:
                        qkv_slab(b, t_o + 4, slabs)
                gram_chunk(b, b * T + T - 1)
                gs = ev.tile([48, 384], F32, tag="gs")
                nc.vector.tensor_copy(gs[:], gps[b][:])
                nc.sync.dma_start(
                    out=gram_in[8 * b:8 * (b + 1)].rearrange(
                        "g c d -> c g d"),
                    in_=gs[:].rearrange("c (g d) -> c g d", g=8))
                nc.gpsimd.collective_compute(
                    "AllReduce", AL.add,
                    replica_groups=[list(range(NCORES))],
                    ins=[gram_in[8 * b:8 * (b + 1)]],
                    outs=[gram_out[8 * b:8 * (b + 1)]])
                bts = attn_batch(b)
                if b == 0:
                    bts0 = bts
                    av_proj(bts, range(0, 5))
                else:
                    # deferred batch-0 chunks fill the batch-1
                    # AllReduce/softmax latency window
                    av_proj(bts0, range(5, T))
                    av_proj(bts, range(T, 2 * T))

    nc.compile()
    return nc


def _prep_inputs(x, qkv_w, qkv_b, dw_w, dw_b, temperature, proj_w, proj_b):
    """Host-side prep: per-core padded fp16 slabs + shared weights."""
    x = np.asarray(x, np.float32)
    b_, c_, t_, h_, w_ = x.shape  # 2, 192, 8, 64, 64
    qkv_w2 = np.asarray(qkv_w, np.float32).reshape(C3, DIM)
    dw_w2 = np.asarray(dw_w, np.float32).reshape(C3, 27)
    proj_w2 = np.asarray(proj_w, np.float32).reshape(DIM, DIM)
    # permute qkv channels: [q_h0, k_h0, q_h1, k_h1, ..., v] so each head's
    # (q,k) columns are adjacent after transpose (contiguous gram operands)
    perm = []
    for h in range(HEADS):
        perm.extend(range(HD * h, HD * (h + 1)))          # q_h
        perm.extend(range(DIM + HD * h, DIM + HD * (h + 1)))  # k_h
    perm.extend(range(2 * DIM, 3 * DIM))                  # v unchanged
    perm = np.array(perm)
    qkv_w2 = qkv_w2[perm]
    dw_w2 = dw_w2[perm]
    qkv_b = np.asarray(qkv_b, np.float32)[perm]
    dw_b = np.asarray(dw_b, np.float32)[perm]

    import ml_dtypes
    FP8 = ml_dtypes.float8_e4m3

    wqT = np.ascontiguousarray(qkv_w2.T).astype(np.float16)  # [192, 576]
    qkvb_h = np.zeros((128, 5), np.float32)
    dwb_h = np.zeros((128, 5), np.float32)
    for mi, (mo, mc) in enumerate(MTILES):
        s = ASCALE if mi < 3 else 1.0  # qk slab evicted as fp8(ASCALE*psum)
        qkvb_h[:mc, mi] = np.asarray(qkv_b, np.float32)[mo:mo + mc] * s
        dwb_h[:mc, mi] = np.asarray(dw_b, np.float32)[mo:mo + mc]

    # fp8 DoubleRow diag-pair tiles for q/k (values WSCALE*d, fp8-rounded)
    tap_i = {tap: i for i, tap in enumerate(TAPS)}
    qkd = np.zeros((128, 3 * 3 * NQK_TILES * 256), FP8)
    d8 = (WSCALE * dw_w2).astype(FP8)  # [576, 27]
    rng = np.arange(128)
    for mi in range(3):
        mo = 128 * mi
        for dti, dt in enumerate((-1, 0, 1)):
            for jj in range(NQK_TILES):
                dwv = jj % 3 - 1
                ti = (mi * 3 + dti) * NQK_TILES + jj
                base = 256 * ti
                if jj < 3:
                    qkd[rng, base + rng] = d8[mo + rng, tap_i[(dt, -1, dwv)]]
                    qkd[rng, base + 128 + rng] = d8[mo + rng,
                                                    tap_i[(dt, 1, dwv)]]
                else:
                    qkd[rng, base + rng] = d8[mo + rng, tap_i[(dt, 0, dwv)]]

    # exact fp16 diag tiles for v
    vd3_h = np.zeros((128, 27 * 128), np.float16)
    vd4_h = np.zeros((64, 27 * 64), np.float16)
    for ti in range(27):
        vd3_h[rng, 128 * ti + rng] = dw_w2[384 + rng, ti].astype(np.float16)
        r64 = np.arange(64)
        vd4_h[r64, 64 * ti + r64] = dw_w2[512 + r64, ti].astype(np.float16)

    # per-channel f32 v-diag columns for the Pool-engine taps
    vdws_h = np.zeros((128, 54), np.float32)
    vdws_h[:, 0:27] = dw_w2[384:512]
    vdws_h[:64, 27:54] = dw_w2[512:576]

    # fp8 q/k 1x1-conv weights [96, 2, 128] per qk mtile (lhsT layout:
    # W[p, j, m] = qkv_w[out=mo+m, in=p+96j])
    wq8_h = np.zeros((96, 3 * 256), FP8)
    for mi in range(3):
        for j in range(2):
            blk = qkv_w2[128 * mi:128 * (mi + 1), 96 * j:96 * (j + 1)].T
            wq8_h[:, 256 * mi + 128 * j:256 * mi + 128 * (j + 1)] = \
                blk.astype(FP8)
    # proj lhsT with contraction padded 192->2x96 (no padding needed: 96*2)
    projwT_h = np.ascontiguousarray(proj_w2.T).astype(np.float16)  # [192,192]
    projb_h = np.zeros((128, 2), np.float32)
    projb_h[:128, 0] = np.asarray(proj_b, np.float32)[0:128]
    projb_h[:64, 1] = np.asarray(proj_b, np.float32)[128:192]
    temp_h = np.repeat(np.asarray(temperature, np.float32).reshape(HEADS, 1),
                       2, axis=1)  # [head, batch]

    in_maps = []
    for i in range(NCORES):
        # padded slab [b, t10, h10, w66], h rows 8i-1 .. 8i+9 clamped->zero
        xs = np.zeros((b_, TP, HP, XW, c_), np.float32)
        hlo, hhi = 8 * i - 1, 8 * i + 9
        slo, shi = max(0, hlo), min(h_, hhi)
        # x [b,c,t,h,w] -> [b,t,h,w,c]
        xt = x[:, :, :, slo:shi, :].transpose(0, 2, 3, 4, 1)
        xs[:, 1:9, (slo - hlo):(slo - hlo) + (shi - slo), 1:65, :] = xt
        xflat = xs.reshape(b_ * TP * HP * XW, c_)
        x16 = np.ascontiguousarray(xflat.T).astype(np.float16)
        x8_h = np.ascontiguousarray(
            xflat.T.reshape(2, 96, NPADTOK).transpose(1, 0, 2)
            .reshape(96, 2 * NPADTOK)).astype(FP8)
        in_maps.append({
            "x16": x16, "x8": x8_h, "wq8d": wq8_h, "vdws": vdws_h,
            "qkvwT": wqT, "qkvb": qkvb_h, "qkdiag": qkd,
            "vdiag3": vd3_h, "vdiag4": vd4_h,
            "dwb": dwb_h, "projwT": projwT_h, "projb": projb_h,
            "temp": temp_h,
        })
    return in_maps


def _get_runner():
    """Build once; return a persistent sharded-jit callable (the per-call
    closure in bass2jax.run_bass_via_pjrt defeats jax's jit cache)."""
    if "runner" in _CACHE:
        return _CACHE["runner"]
    import jax
    for flag, val in [("jax_compilation_cache_dir", "/tmp/jax_kernel_cache"),
                      ("jax_persistent_cache_min_compile_time_secs", 1.0),
                      ("jax_persistent_cache_min_entry_size_bytes", 0)]:
        try:
            jax.config.update(flag, val)
        except Exception:
            pass
    import jax.numpy as jnp
    from jax.sharding import Mesh, PartitionSpec
    from jax.experimental.shard_map import shard_map
    import concourse.mybir as mybir
    from concourse import bass2jax

    nc = _build()
    bass2jax.install_neuronx_cc_hook()

    partition_name = (nc.partition_id_tensor.name
                      if nc.partition_id_tensor else None)
    in_names, out_names, out_avals, zero_shapes = [], [], [], []
    for alloc in nc.m.functions[0].allocations:
        if not isinstance(alloc, mybir.MemoryLocationSet):
            continue
        name = alloc.memorylocations[0].name
        if alloc.kind == "ExternalInput":
            if name != partition_name:
                in_names.append(name)
        elif alloc.kind == "ExternalOutput":
            shape = tuple(alloc.tensor_shape)
            dtype = mybir.dt.np(alloc.dtype)
            out_names.append(name)
            out_avals.append(jax.core.ShapedArray(shape, dtype))
            zero_shapes.append((shape, dtype))
    n_params = len(in_names)
    all_names = in_names + out_names
    if partition_name is not None:
        all_names.append(partition_name)

    def _body(*args):
        operands = list(args)
        if partition_name is not None:
            operands.append(bass2jax.partition_id_tensor())
        outs = bass2jax._bass_exec_p.bind(
            *operands, out_avals=tuple(out_avals), in_names=tuple(all_names),
            out_names=tuple(out_names), lowering_input_output_aliases=(),
            sim_require_finite=True, sim_require_nnan=True, nc=nc)
        return tuple(outs)

    devices = jax.devices()[:NCORES]
    mesh = Mesh(np.asarray(devices), ("core",))
    n_outs = len(out_names)
    sharded = jax.jit(
        shard_map(_body, mesh=mesh,
                  in_specs=(PartitionSpec("core"),) * (n_params + n_outs),
                  out_specs=(PartitionSpec("core"),) * n_outs,
                  check_rep=False),
        donate_argnums=tuple(range(n_params, n_params + n_outs)),
        keep_unused=True)

    def run(in_maps):
        concat_in = [np.concatenate([in_maps[c][nm] for c in range(NCORES)],
                                    axis=0) for nm in in_names]
        concat_zeros = [np.zeros((NCORES * s[0], *s[1:]), dt)
                        for s, dt in zero_shapes]
        out_arrs = sharded(*concat_in, *concat_zeros)
        return [
            {nm: np.asarray(out_arrs[i]).reshape(NCORES, *out_avals[i].shape)[c]
             for i, nm in enumerate(out_names)}
            for c in range(NCORES)]

    _CACHE["runner"] = run
    return run


def kernel(x, qkv_w, qkv_b, dw_w, dw_b, temperature, proj_w, proj_b):
    run = _get_runner()
    in_maps = _prep_inputs(x, qkv_w, qkv_b, dw_w, dw_b, temperature,
                           proj_w, proj_b)
    results = run(in_maps)
    b_, c_, t_, h_, w_ = np.asarray(x).shape
    outf = np.empty((b_, c_, t_, h_, w_), np.float32)
    for i in range(NCORES):
        o = results[i]["out"].reshape(c_, b_, t_, H, w_)
        outf[:, :, :, 8 * i:8 * i + 8, :] = o.transpose(1, 0, 2, 3, 4)
    return outf

